# revision 1
# baseline (speedup 1.0000x reference)
"""Trainium2 Bass kernel for nn_Decoder_20486994002617.

8-core tensor-parallel 2-layer llama-style decoder with ragged token-merge
(handled on host), returning the masked-mean cross-entropy loss.

Device layout choices:
  - h (residual) lives in SBUF as [128 part, 8 seq-tiles, 4096] bf16.
  - RMSNorm weights are folded into the consumer weight matrices on host,
    so the device norm is x * rsqrt(mean(x^2)+eps) only; the multiply by
    the per-row factor is fused into the seq->feature transpose as a
    matmul against diag(factor).
  - Attention: heads sharded 4 q-heads + 1 kv-head per core (GQA groups
    align), scores/softmax per (head, 128-row tile), causal mask added via
    an extra accumulating matmul (I.T @ cmask), attn probs transposed back
    through the PE with diag(1/sumexp) fused.
  - MLP: intermediate dim sharded 1376/core, padded to 1408 = 11*128.
  - lm_head: vocab sharded 4000/core; softmax stats (row max, sum-exp) are
    AllReduce'd; the target logit is computed via a host-gathered column
    matrix (wsel) so no device gather is needed.
Outputs per core: gmax [128,8] f32, gsum [128,8] f32, tlog [1,1024] f32.
Host finishes: ce = gmax + log(gsum) - tlog; loss = masked mean.
"""
import numpy as np
import ml_dtypes

from contextlib import ExitStack

import concourse.bass as bass
import concourse.bacc as bacc
import concourse.mybir as mybir
import concourse.tile as tile
from concourse.bass_utils import run_bass_kernel_spmd

F32 = mybir.dt.float32
BF16 = mybir.dt.bfloat16
AF = mybir.ActivationFunctionType
ALU = mybir.AluOpType
AX = mybir.AxisListType

H, HD, NH, NKV = 4096, 128, 32, 8
L, V, S, I = 2, 32000, 1024, 11008
EPS, THETA = 1e-6, 10000.0
NC_ = 8          # cores
IPC = I // NC_   # 1376
IP = 1408        # padded intermediate per core = 11 * 128
VS = V // NC_    # 4000 vocab per core
NEG = -1e9

bf16 = ml_dtypes.bfloat16

last_run_info = {}
_cache = {}


# ----------------------------------------------------------------- device --

def _norm_transpose(nc, pools, h_ap, dst, ident_sb, uid, nt_tag="nt_ps", nt_bufs=2):
    """dst[:, k, :] (32 chunks of [128,128]) = normalized transpose of
    h_ap ([128 seq rows, 4096]). dst free dims must be (32, 128)."""
    small, ntmp, psum = pools
    ssq = small.tile([128, 1], F32, tag="nt_ssq", bufs=2, name=f"ssq_{uid}")
    # Square scratch output goes into dst (overwritten by the transpose after)
    nc.scalar.activation(dst, h_ap.rearrange("p (k m) -> p k m", k=32),
                         AF.Square, accum_out=ssq[:])
    var = small.tile([128, 1], F32, tag="nt_var", bufs=2, name=f"var_{uid}")
    nc.vector.tensor_scalar(var[:], ssq[:], 1.0 / H, EPS, op0=ALU.mult, op1=ALU.add)
    std = small.tile([128, 1], F32, tag="nt_std", bufs=2, name=f"std_{uid}")
    nc.scalar.sqrt(std[:], var[:])
    fac = small.tile([128, 1], F32, tag="nt_fac", bufs=2, name=f"fac_{uid}")
    nc.vector.reciprocal(fac[:], std[:])
    diag = ntmp.tile([128, 128], BF16, tag="nt_diag", bufs=2, name=f"diag_{uid}")
    nc.vector.tensor_scalar_mul(diag[:], ident_sb[:], fac[:])
    for kk in range(8):
        pnt = psum.tile([128, 512], F32, tag=nt_tag, bufs=nt_bufs,
                        name=f"pnt_{uid}_{kk}")
        for j in range(4):
            k = kk * 4 + j
            nc.tensor.matmul(pnt[:, j * 128:(j + 1) * 128],
                             h_ap[:, k * 128:(k + 1) * 128], diag[:],
                             start=True, stop=True)
        nc.any.tensor_copy(dst[:, kk * 4:(kk + 1) * 4, :],
                           pnt[:].rearrange("p (j m) -> p j m", j=4))


def _rope(nc, pools, ps, cos_ap, sf_ap, out, nheads, i):
    """out (bf16 [128, nheads*128]) = rope(ps) with ps a psum slice."""
    small, ntmp, psum = pools
    n = nheads * 128
    t1 = ntmp.tile([128, 512], F32, tag="rope_t1", bufs=1, name=f"t1_{i}_{nheads}")
    t2 = ntmp.tile([128, 512], F32, tag="rope_t2", bufs=1, name=f"t2_{i}_{nheads}")
    nc.vector.tensor_mul(t1[:, :n], ps, cos_ap)
    for hh in range(nheads):
        b = hh * 128
        nc.vector.tensor_mul(t2[:, b:b + 64], ps[:, b + 64:b + 128],
                             sf_ap[:, b:b + 64])
        nc.vector.tensor_mul(t2[:, b + 64:b + 128], ps[:, b:b + 64],
                             sf_ap[:, b + 64:b + 128])
    nc.vector.tensor_add(out[:], t1[:, :n], t2[:, :n])


def build_nc():
    nc = bacc.Bacc("TRN2", target_bir_lowering=False, debug=False,
                   num_devices=NC_)

    din = {}
    def dram_in(name, shape):
        din[name] = nc.dram_tensor(name, shape, BF16, kind="ExternalInput")
        return din[name]

    h0_d = dram_in("h0", [S, H])
    cos4_d = dram_in("cos4", [S, 512])
    sf4_d = dram_in("sf4", [S, 512])
    ident_d = dram_in("ident", [128, 128])
    cmask_d = dram_in("cmask", [128, 128])
    ones_d = dram_in("ones", [128, 1])
    for l in range(L):
        dram_in(f"qw{l}", [H, 512])
        dram_in(f"kvw{l}", [H, 256])
        dram_in(f"ow{l}", [512, H])
        dram_in(f"gw{l}", [H, IP])
        dram_in(f"uw{l}", [H, IP])
        dram_in(f"dw{l}", [IP, H])
    lmw_d = dram_in("lmw", [8, H, VS // 8])
    wsel_d = dram_in("wsel", [H, S])

    gmax_o = nc.dram_tensor("gmax_o", [128, 8], F32, kind="ExternalOutput")
    gsum_o = nc.dram_tensor("gsum_o", [128, 8], F32, kind="ExternalOutput")
    tlog_o = nc.dram_tensor("tlog_o", [1, S], F32, kind="ExternalOutput")

    rg = [list(range(NC_))]

    with tile.TileContext(nc) as tc:
        with (
            tc.tile_pool(name="pconst", bufs=1) as pconst,
            tc.tile_pool(name="psmall", bufs=1) as psmall,
            tc.tile_pool(name="pdram", bufs=1, space="DRAM") as pdram,
        ):
            ident_sb = pconst.tile([128, 128], BF16)
            cmask_sb = pconst.tile([128, 128], BF16)
            ones_sb = pconst.tile([128, 1], BF16)
            cos4_sb = pconst.tile([128, 8, 512], BF16)
            sf4_sb = pconst.tile([128, 8, 512], BF16)
            nc.sync.dma_start(ident_sb[:], ident_d.ap())
            nc.sync.dma_start(cmask_sb[:], cmask_d.ap())
            nc.sync.dma_start(ones_sb[:], ones_d.ap())
            for i in range(8):
                nc.sync.dma_start(cos4_sb[:, i, :], cos4_d.ap()[i * 128:(i + 1) * 128, :])
                nc.sync.dma_start(sf4_sb[:, i, :], sf4_d.ap()[i * 128:(i + 1) * 128, :])

            hstack = ExitStack()
            phh = hstack.enter_context(tc.tile_pool(name="phh", bufs=1))
            if True:
                h_sb = phh.tile([128, 8, H], BF16)
                for i in range(8):
                    nc.sync.dma_start(h_sb[:, i, :], h0_d.ap()[i * 128:(i + 1) * 128, :])

                ar_ins, ar_outss, ar2_ins, ar2_outss = [], [], [], []
                for l in range(L):
                    ar_ins.append(pdram.tile([S, H], BF16, tag=f"ar_in_{l}",
                                             name=f"ar_in_{l}"))
                    ar_outss.append([pdram.tile([512, H], BF16, addr_space="Shared",
                                                tag=f"ar_out_{l}_{c}",
                                                name=f"ar_out_{l}_{c}")
                                     for c in range(2)])
                    ar2_ins.append(pdram.tile([S, H], BF16, tag=f"ar2_in_{l}",
                                              name=f"ar2_in_{l}"))
                    ar2_outss.append([pdram.tile([512, H], BF16, addr_space="Shared",
                                                 tag=f"ar2_out_{l}_{c}",
                                                 name=f"ar2_out_{l}_{c}")
                                      for c in range(2)])

                for l in range(L):
                    # ======== attention: per-tile qkv -> heads -> o-proj ====
                    with (
                        tc.tile_pool(name="pal", bufs=1) as pal,
                        tc.tile_pool(name="paps", bufs=1, space="PSUM") as paps,
                    ):
                        kT_sb = pal.tile([128, S], BF16)
                        v_sb = pal.tile([128, 8, 128], BF16)
                        ar_in = ar_ins[l]
                        ar_outs = ar_outss[l]
                        pools = (psmall, pal, paps)
                        wq_sb = pal.tile([128, 32, 512], BF16)
                        wkv_sb = pal.tile([128, 32, 256], BF16)
                        ow_sb = pal.tile([128, 4, H], BF16)
                        nc.sync.dma_start(
                            wq_sb[:], din[f"qw{l}"].ap().rearrange("(k p) n -> p k n", p=128))
                        nc.sync.dma_start(
                            wkv_sb[:], din[f"kvw{l}"].ap().rearrange("(k p) n -> p k n", p=128))
                        nc.sync.dma_start(
                            ow_sb[:], din[f"ow{l}"].ap().rearrange("(t p) n -> p t n", p=128))
                        for i in range(8):
                            if l > 0:
                                rt = pal.tile([128, H], BF16, tag="resprev",
                                              bufs=1, name=f"resprev_{l}_{i}")
                                nc.sync.dma_start(
                                    rt[:],
                                    ar2_outss[l - 1][i // 4][(i % 4) * 128:(i % 4 + 1) * 128, :])
                                nc.vector.tensor_add(h_sb[:, i, :], h_sb[:, i, :], rt[:])
                            qT_sb = pal.tile([128, 4, 128], BF16, tag="qT",
                                             bufs=2, name=f"qT_{l}_{i}")
                            oT_sb = pal.tile([128, 4, 128], BF16, tag="oT",
                                             bufs=2, name=f"oT_{l}_{i}")
                            xnt = pal.tile([128, 32, 128], BF16, tag="xnt",
                                           bufs=1, name=f"xnt_{l}_{i}")
                            _norm_transpose(nc, pools, h_sb[:, i, :], xnt, ident_sb,
                                            f"a{l}_{i}", nt_bufs=1)
                            psq = paps.tile([128, 512], F32, tag="psq", bufs=1,
                                            name=f"psq_{l}_{i}")
                            pskv = paps.tile([128, 256], F32, tag="pskv", bufs=1,
                                             name=f"pskv_{l}_{i}")
                            for k in range(32):
                                nc.tensor.matmul(psq[:], xnt[:, k, :], wq_sb[:, k, :],
                                                 start=(k == 0), stop=(k == 31))
                                nc.tensor.matmul(pskv[:], xnt[:, k, :], wkv_sb[:, k, :],
                                                 start=(k == 0), stop=(k == 31))
                            q_rot = pal.tile([128, 512], BF16, tag="q_rot", bufs=2,
                                             name=f"qr_{l}_{i}")
                            k_rot = pal.tile([128, 128], BF16, tag="k_rot", bufs=2,
                                             name=f"kr_{l}_{i}")
                            _rope(nc, pools, psq[:], cos4_sb[:, i, :], sf4_sb[:, i, :],
                                  q_rot, 4, f"{l}_{i}")
                            _rope(nc, pools, pskv[:, 0:128], cos4_sb[:, i, 0:128],
                                  sf4_sb[:, i, 0:128], k_rot, 1, f"{l}_{i}")
                            nc.any.tensor_copy(v_sb[:, i, :], pskv[:, 128:256])
                            for hh in range(4):
                                ptr = paps.tile([128, 512], F32, tag="mix", bufs=2,
                                                name=f"ptrq_{l}_{i}_{hh}")
                                nc.tensor.matmul(ptr[:, :128], q_rot[:, hh * 128:(hh + 1) * 128],
                                                 ident_sb[:], start=True, stop=True)
                                nc.any.tensor_copy(qT_sb[:, hh, :], ptr[:, :128])
                            ptrk = paps.tile([128, 512], F32, tag="mix", bufs=2,
                                             name=f"ptrk_{l}_{i}")
                            nc.tensor.matmul(ptrk[:, :128], k_rot[:], ident_sb[:],
                                             start=True, stop=True)
                            nc.any.tensor_copy(kT_sb[:, i * 128:(i + 1) * 128], ptrk[:, :128])
                            n2 = 128 * (i + 1)
                            for hh in range(4):
                                pss = paps.tile([128, 1024], F32, tag="pss", bufs=1,
                                                name=f"pss_{l}_{hh}_{i}")
                                lhs_q = qT_sb[:, hh, :]
                                c0 = 0
                                while c0 < n2 - 128:
                                    N = min(512, n2 - 128 - c0)
                                    nc.tensor.matmul(pss[:, c0:c0 + N], lhs_q,
                                                     kT_sb[:, c0:c0 + N],
                                                     start=True, stop=True)
                                    c0 += N
                                nc.tensor.matmul(pss[:, n2 - 128:n2], lhs_q,
                                                 kT_sb[:, n2 - 128:n2],
                                                 start=True, stop=False)
                                nc.tensor.matmul(pss[:, n2 - 128:n2], ident_sb[:],
                                                 cmask_sb[:], start=False, stop=True)
                                mx = psmall.tile([128, 1], F32, tag="mx", bufs=2,
                                                 name=f"mx_{l}_{hh}_{i}")
                                nc.vector.tensor_reduce(mx[:], pss[:, :n2], axis=AX.X,
                                                        op=ALU.max)
                                negm = psmall.tile([128, 1], F32, tag="negm", bufs=2,
                                                   name=f"negm_{l}_{hh}_{i}")
                                nc.vector.tensor_scalar_mul(negm[:], mx[:], -1.0)
                                sume = psmall.tile([128, 1], F32, tag="sume", bufs=2,
                                                   name=f"sume_{l}_{hh}_{i}")
                                exp_sb = pal.tile([128, 1024], BF16, tag="exp", bufs=1,
                                                  name=f"exp_{l}_{hh}_{i}")
                                nc.scalar.activation(exp_sb[:, :n2], pss[:, :n2], AF.Exp,
                                                     bias=negm[:], accum_out=sume[:])
                                rec = psmall.tile([128, 1], F32, tag="rec", bufs=2,
                                                  name=f"rec_{l}_{hh}_{i}")
                                nc.vector.reciprocal(rec[:], sume[:])
                                diag_r = pal.tile([128, 128], BF16, tag="diag_r", bufs=2,
                                                  name=f"diagr_{l}_{hh}_{i}")
                                nc.vector.tensor_scalar_mul(diag_r[:], ident_sb[:], rec[:])
                                atcol = pal.tile([128, 8, 128], BF16, tag="atcol", bufs=1,
                                                 name=f"atcol_{l}_{hh}_{i}")
                                for j in range(i + 1):
                                    pat = paps.tile([128, 512], F32, tag="mix", bufs=2,
                                                    name=f"pat_{l}_{hh}_{i}_{j}")
                                    nc.tensor.matmul(pat[:, :128], exp_sb[:, j * 128:(j + 1) * 128],
                                                     diag_r[:], start=True, stop=True)
                                    nc.any.tensor_copy(atcol[:, j, :], pat[:, :128])
                                pso = paps.tile([128, 128], F32, tag="pso", bufs=1,
                                                name=f"pso_{l}_{hh}_{i}")
                                for j in range(i + 1):
                                    nc.tensor.matmul(pso[:], v_sb[:, j, :], atcol[:, j, :],
                                                     start=(j == 0), stop=(j == i))
                                nc.any.tensor_copy(oT_sb[:, hh, :], pso[:])
                            ob = pal.tile([128, H], BF16, tag="ob", bufs=1,
                                          name=f"ob_{l}_{i}")
                            for n in range(8):
                                pps = paps.tile([128, 512], F32, tag="mix", bufs=2,
                                                name=f"pop_{l}_{i}_{n}")
                                for t in range(4):
                                    nc.tensor.matmul(pps[:], oT_sb[:, t, :],
                                                     ow_sb[:, t, n * 512:(n + 1) * 512],
                                                     start=(t == 0), stop=(t == 3))
                                nc.any.tensor_copy(ob[:, n * 512:(n + 1) * 512], pps[:])
                            nc.sync.dma_start(ar_in[i * 128:(i + 1) * 128, :], ob[:])
                            if i == 3:
                                nc.gpsimd.collective_compute(
                                    "AllReduce", ALU.add, replica_groups=rg,
                                    ins=[ar_in[0:512, :].opt()], outs=[ar_outs[0].opt()])
                        nc.gpsimd.collective_compute(
                            "AllReduce", ALU.add, replica_groups=rg,
                            ins=[ar_in[512:1024, :].opt()], outs=[ar_outs[1].opt()])

                    # ===== MLP: per-half gate/up -> down -> AR2 =============
                    with (
                        tc.tile_pool(name="pml", bufs=1) as pml,
                        tc.tile_pool(name="pmps", bufs=1, space="PSUM") as pmps,
                    ):
                        ar2_in = ar2_ins[l]
                        ar2_outs = ar2_outss[l]
                        for ig in range(2):
                            with tc.tile_pool(name="pgu", bufs=1) as pgu:
                                pools = (psmall, pgu, pmps)
                                yt_sb = pml.tile([128, 11, 512], BF16, tag="yt",
                                                 bufs=2, name=f"yt_{l}_{ig}")
                                xnts = []
                                for ii in range(4):
                                    i = ig * 4 + ii
                                    rt = pgu.tile([128, H], BF16, tag="resat",
                                                  bufs=1, name=f"resat_{l}_{i}")
                                    nc.sync.dma_start(
                                        rt[:], ar_outs[ig][ii * 128:(ii + 1) * 128, :])
                                    nc.vector.tensor_add(h_sb[:, i, :], h_sb[:, i, :], rt[:])
                                    xnt = pgu.tile([128, 32, 128], BF16, tag="xnt2",
                                                   bufs=4, name=f"xnt2_{l}_{i}")
                                    _norm_transpose(nc, pools, h_sb[:, i, :], xnt, ident_sb,
                                                    f"m{l}_{i}", nt_tag="mlpps", nt_bufs=4)
                                    xnts.append(xnt)
                                gu = {}
                                for wname, tag in ((f"gw{l}", "g"), (f"uw{l}", "u")):
                                    outs = [pgu.tile([128, IP], BF16, tag=tag, bufs=4,
                                                     name=f"{tag}_{l}_{ig}_{ii}")
                                            for ii in range(4)]
                                    gu[tag] = outs
                                    for nb in range(3):
                                        NB = 512 if nb < 2 else IP - 1024
                                        pg = [pmps.tile([128, 512], F32, tag="mlpps", bufs=4,
                                                        name=f"pg_{l}_{ig}_{tag}_{nb}_{ii}")
                                              for ii in range(4)]
                                        for kp in range(8):
                                            wt = pgu.tile([128, 4, 512], BF16, tag="wstream",
                                                          bufs=2,
                                                          name=f"wt_{l}_{ig}_{tag}_{nb}_{kp}")
                                            nc.sync.dma_start(
                                                wt[:, :, :NB],
                                                din[wname].ap()[kp * 512:(kp + 1) * 512,
                                                                nb * 512:nb * 512 + NB]
                                                .rearrange("(j p) n -> p j n", p=128))
                                            for jk in range(4):
                                                k = kp * 4 + jk
                                                for ii in range(4):
                                                    nc.tensor.matmul(pg[ii][:, :NB],
                                                                     xnts[ii][:, k, :],
                                                                     wt[:, jk, :NB],
                                                                     start=(k == 0), stop=(k == 31))
                                        for ii in range(4):
                                            nc.any.tensor_copy(
                                                outs[ii][:, nb * 512:nb * 512 + NB],
                                                pg[ii][:, :NB])
                                for ii in range(4):
                                    i = ig * 4 + ii
                                    ysil = pgu.tile([128, IP], BF16, tag="ysil", bufs=2,
                                                    name=f"ysil_{l}_{i}")
                                    nc.scalar.activation(ysil[:], gu["g"][ii][:], AF.Silu)
                                    y = gu["u"][ii]
                                    nc.vector.tensor_mul(y[:], ysil[:], y[:])
                                    for tq in range(3):
                                        ts = [tq * 4 + j for j in range(4) if tq * 4 + j < 11]
                                        ptr = pmps.tile([128, 512], F32, tag="mlpps", bufs=4,
                                                        name=f"ytr_{l}_{i}_{tq}")
                                        for jj, t in enumerate(ts):
                                            nc.tensor.matmul(ptr[:, jj * 128:(jj + 1) * 128],
                                                             y[:, t * 128:(t + 1) * 128],
                                                             ident_sb[:], start=True, stop=True)
                                        nc.any.tensor_copy(
                                            yt_sb[:, ts[0]:ts[0] + len(ts),
                                                  ii * 128:(ii + 1) * 128],
                                            ptr[:, :len(ts) * 128].rearrange(
                                                "p (j m) -> p j m", j=len(ts)))
                                for n in range(8):
                                    pd = [pmps.tile([128, 512], F32, tag=f"pd{ii}", bufs=1,
                                                    name=f"pd_{l}_{ig}_{n}_{ii}")
                                          for ii in range(4)]
                                    for tp in range(3):
                                        nt = 4 if tp < 2 else 3
                                        dwt = pgu.tile([128, 4, 512], BF16, tag="dwstream",
                                                       bufs=2, name=f"dwt_{l}_{ig}_{n}_{tp}")
                                        nc.sync.dma_start(
                                            dwt[:, :nt, :],
                                            din[f"dw{l}"].ap()[tp * 512:tp * 512 + nt * 128,
                                                               n * 512:(n + 1) * 512]
                                            .rearrange("(j p) n -> p j n", p=128))
                                        for jt in range(nt):
                                            t = tp * 4 + jt
                                            for ii in range(4):
                                                nc.tensor.matmul(
                                                    pd[ii][:], yt_sb[:, t, ii * 128:(ii + 1) * 128],
                                                    dwt[:, jt, :], start=(t == 0), stop=(t == 10))
                                    for ii in range(4):
                                        i = ig * 4 + ii
                                        db = pgu.tile([128, 512], BF16, tag="db", bufs=4,
                                                      name=f"db_{l}_{ig}_{n}_{ii}")
                                        nc.any.tensor_copy(db[:], pd[ii][:])
                                        nc.sync.dma_start(
                                            ar2_in[i * 128:(i + 1) * 128,
                                                   n * 512:(n + 1) * 512], db[:])
                                nc.gpsimd.collective_compute(
                                    "AllReduce", ALU.add, replica_groups=rg,
                                    ins=[ar2_in[ig * 512:(ig + 1) * 512, :].opt()],
                                    outs=[ar2_outs[ig].opt()])

                # spill h to DRAM so the h pool can close before lm phase
                hdram = pdram.tile([S, H], BF16)
                for i in range(8):
                    nc.gpsimd.dma_start(hdram[i * 128:(i + 1) * 128, :], h_sb[:, i, :])
            hstack.close()  # release h pool

            # ======================= final norm -> xf ======================
            with tc.tile_pool(name="pxf", bufs=1) as pxf:
                xf_sb = pxf.tile([128, 32, S], BF16)
                with (
                    tc.tile_pool(name="pfn", bufs=1) as pfn,
                    tc.tile_pool(name="pfps", bufs=1, space="PSUM") as pfps,
                ):
                    pools = (psmall, pfn, pfps)
                    for i in range(8):
                        ht = pfn.tile([128, H], BF16, tag="hfin", bufs=2,
                                      name=f"hfin_{i}")
                        nc.gpsimd.dma_start(ht[:], hdram[i * 128:(i + 1) * 128, :])
                        rt = pfn.tile([128, H], BF16, tag="resfin", bufs=2,
                                      name=f"resfin_{i}")
                        nc.sync.dma_start(
                            rt[:], ar2_outss[L - 1][i // 4][(i % 4) * 128:(i % 4 + 1) * 128, :])
                        nc.vector.tensor_add(ht[:], ht[:], rt[:])
                        dst = xf_sb[:, :, i * 128:(i + 1) * 128]
                        _norm_transpose(nc, pools, ht[:], dst, ident_sb, f"f{i}")
                self_lm_phases(nc, tc, psmall, xf_sb, ident_sb, ones_sb,
                               wsel_d, lmw_d, tlog_o, gmax_o, gsum_o, rg)

    nc.compile()
    return nc


def self_lm_phases(nc, tc, psmall, xf_sb, ident_sb, ones_sb, wsel_d, lmw_d,
                   tlog_o, gmax_o, gsum_o, rg):
            if True:
                pass
            with (
                tc.tile_pool(name="ptl", bufs=1) as ptl,
                tc.tile_pool(name="ptps", bufs=1, space="PSUM") as ptps,
            ):
                pt0 = ptps.tile([1, 512], F32)
                pt1 = ptps.tile([1, 512], F32)
                for kp in range(8):
                    ws = ptl.tile([128, 4, S], BF16, tag="wsel", bufs=2, name=f"ws_{kp}")
                    nc.sync.dma_start(
                        ws[:], wsel_d.ap()[kp * 512:(kp + 1) * 512, :]
                        .rearrange("(j p) n -> p j n", p=128))
                    for jk in range(4):
                        k = kp * 4 + jk
                        tm = ptl.tile([128, S], BF16, tag="tm", bufs=2, name=f"tm_{k}")
                        nc.vector.tensor_mul(tm[:], xf_sb[:, k, :], ws[:, jk, :])
                        nc.tensor.matmul(pt0[:], ones_sb[:], tm[:, :512],
                                         start=(k == 0), stop=(k == 31))
                        nc.tensor.matmul(pt1[:], ones_sb[:], tm[:, 512:],
                                         start=(k == 0), stop=(k == 31))
                tl_sb = ptl.tile([1, S], F32)
                nc.any.tensor_copy(tl_sb[:, :512], pt0[:])
                nc.any.tensor_copy(tl_sb[:, 512:], pt1[:])
                nc.sync.dma_start(tlog_o.ap(), tl_sb[:])

            with (
                tc.tile_pool(name="plm", bufs=1) as plm,
                tc.tile_pool(name="plps", bufs=1, space="PSUM") as plps,
                tc.tile_pool(name="pld", bufs=1, space="DRAM") as pld,
            ):
                logits = [plm.tile([128, VS], BF16, tag=f"lg{i}", bufs=1,
                                   name=f"logits_{i}") for i in range(8)]
                for vb in range(8):
                    pl = [plps.tile([128, 500], F32, tag=f"pl{i}", bufs=1,
                                    name=f"pl_{vb}_{i}") for i in range(8)]
                    for kp in range(8):
                        lt = plm.tile([128, 4, 500], BF16, tag="lmw", bufs=4,
                                      name=f"lt_{vb}_{kp}")
                        nc.sync.dma_start(
                            lt[:], lmw_d.ap()[vb, kp * 512:(kp + 1) * 512, :]
                            .rearrange("(j p) n -> p j n", p=128))
                        for jk in range(4):
                            k = kp * 4 + jk
                            for i in range(8):
                                nc.tensor.matmul(pl[i][:], xf_sb[:, k, i * 128:(i + 1) * 128],
                                                 lt[:, jk, :], start=(k == 0), stop=(k == 31))
                    for i in range(8):
                        nc.any.tensor_copy(logits[i][:, vb * 500:(vb + 1) * 500], pl[i][:])

                gmax_sb = plm.tile([128, 8], F32)
                for i in range(8):
                    nc.vector.tensor_reduce(gmax_sb[:, i:i + 1], logits[i][:],
                                            axis=AX.X, op=ALU.max)
                gm_in = pld.tile([128, 8], F32)
                gm_out = pld.tile([128, 8], F32, addr_space="Shared")
                nc.sync.dma_start(gm_in[:], gmax_sb[:])
                nc.gpsimd.collective_compute("AllReduce", ALU.max, replica_groups=rg,
                                             ins=[gm_in.opt()], outs=[gm_out.opt()])
                gm_sb = plm.tile([128, 8], F32)
                nc.sync.dma_start(gm_sb[:], gm_out[:])
                nc.sync.dma_start(gmax_o.ap(), gm_sb[:])
                negg = plm.tile([128, 8], F32)
                nc.vector.tensor_scalar_mul(negg[:], gm_sb[:], -1.0)
                gs_sb = plm.tile([128, 8], F32)
                for i in range(8):
                    scr = plm.tile([128, VS], BF16, tag="scr", bufs=2, name=f"scr_{i}")
                    nc.scalar.activation(scr[:], logits[i][:], AF.Exp,
                                         bias=negg[:, i:i + 1],
                                         accum_out=gs_sb[:, i:i + 1])
                gs_in = pld.tile([128, 8], F32)
                gs_out = pld.tile([128, 8], F32, addr_space="Shared")
                nc.sync.dma_start(gs_in[:], gs_sb[:])
                nc.gpsimd.collective_compute("AllReduce", ALU.add, replica_groups=rg,
                                             ins=[gs_in.opt()], outs=[gs_out.opt()])
                gsf_sb = plm.tile([128, 8], F32)
                nc.sync.dma_start(gsf_sb[:], gs_out[:])
                nc.sync.dma_start(gsum_o.ap(), gsf_sb[:])


# ------------------------------------------------------------------- host --

def host_prep(inputs):
    inp = {k: np.asarray(v) for k, v in inputs.items()}
    embed = inp["embed"].astype(np.float32)
    ids = inp["input_ids"].reshape(-1).astype(np.int64)
    labels = inp["labels"].reshape(-1).astype(np.int64)

    h = embed[ids]
    cw = inp["conv_w"].astype(np.float32)
    logit = h[:-1] @ cw[0, :H] + h[1:] @ cw[0, H:] + np.float32(inp["conv_b"][0])
    mask = logit > 0
    m = np.concatenate([mask, [False]])
    hn = np.where(m[:, None], 0.5 * (h + np.roll(h, -1, axis=0)), h)
    keep = np.concatenate([[True], ~mask])
    order = np.argsort(~keep, kind="stable")
    h0 = hn[order]
    lab = labels[order]
    valid_len = int(keep.sum())

    inv = 1.0 / (THETA ** (np.arange(0, HD, 2, dtype=np.float32) / HD))
    t = np.arange(S, dtype=np.float32)
    freqs = np.outer(t, inv)
    emb = np.concatenate([freqs, freqs], -1)
    cos, sin = np.cos(emb), np.sin(emb)
    sinflip = np.concatenate([-sin[:, :HD // 2], sin[:, HD // 2:]], -1)
    cos4 = np.tile(cos, (1, 4)).astype(bf16)
    sinflip4 = np.tile(sinflip, (1, 4)).astype(bf16)

    ident = np.eye(128, dtype=bf16)
    cmask = np.where(np.arange(128)[None, :] > np.arange(128)[:, None],
                     np.float32(NEG), np.float32(0)).astype(bf16)
    ones = np.ones((128, 1), dtype=bf16)

    ln1 = inp["ln1_w"].astype(np.float32)
    ln2 = inp["ln2_w"].astype(np.float32)
    normw = inp["norm_w"].astype(np.float32)
    qsc = np.float32(1.0 / np.sqrt(HD))
    lm_folded = normw[:, None] * inp["lm_head_w"].astype(np.float32)
    tgt = np.concatenate([lab[1:], [0]]).astype(np.int64)
    wsel = np.ascontiguousarray(lm_folded[:, tgt]).astype(bf16)

    common = dict(h0=h0.astype(bf16), cos4=cos4, sf4=sinflip4, ident=ident,
                  cmask=cmask, ones=ones, wsel=wsel)
    in_maps = []
    for c in range(NC_):
        mcore = dict(common)
        for l in range(L):
            qw = ln1[l][:, None] * inp["q_w"][l].astype(np.float32) * qsc
            kw = ln1[l][:, None] * inp["k_w"][l].astype(np.float32)
            vw = ln1[l][:, None] * inp["v_w"][l].astype(np.float32)
            gw = ln2[l][:, None] * inp["gate_w"][l].astype(np.float32)
            uw = ln2[l][:, None] * inp["up_w"][l].astype(np.float32)
            dw = inp["down_w"][l].astype(np.float32)
            gws = np.zeros((H, IP), np.float32)
            uws = np.zeros((H, IP), np.float32)
            dws = np.zeros((IP, H), np.float32)
            gws[:, :IPC] = gw[:, c * IPC:(c + 1) * IPC]
            uws[:, :IPC] = uw[:, c * IPC:(c + 1) * IPC]
            dws[:IPC] = dw[c * IPC:(c + 1) * IPC]
            mcore[f"qw{l}"] = np.ascontiguousarray(qw[:, c * 512:(c + 1) * 512]).astype(bf16)
            mcore[f"kvw{l}"] = np.concatenate(
                [kw[:, c * 128:(c + 1) * 128], vw[:, c * 128:(c + 1) * 128]],
                1).astype(bf16)
            mcore[f"ow{l}"] = np.ascontiguousarray(
                inp["o_w"][l][c * 512:(c + 1) * 512].astype(np.float32)).astype(bf16)
            mcore[f"gw{l}"] = gws.astype(bf16)
            mcore[f"uw{l}"] = uws.astype(bf16)
            mcore[f"dw{l}"] = dws.astype(bf16)
        lmc = lm_folded[:, c * VS:(c + 1) * VS]
        mcore["lmw"] = np.ascontiguousarray(
            lmc.reshape(H, 8, VS // 8).transpose(1, 0, 2)).astype(bf16)
        in_maps.append(mcore)

    return in_maps, valid_len


def kernel(**inputs) -> np.ndarray:
    in_maps, valid_len = host_prep(inputs)
    if "nc" not in _cache:
        _cache["nc"] = build_nc()
    nc = _cache["nc"]
    res = run_bass_kernel_spmd(nc, in_maps, list(range(NC_)),
                               **last_run_info.get("run_kwargs", {}))
    last_run_info["res"] = res
    out = res.results[0]
    gmax = out["gmax_o"].transpose(1, 0).reshape(S).astype(np.float64)
    gsum = out["gsum_o"].transpose(1, 0).reshape(S).astype(np.float64)
    tlog = out["tlog_o"].reshape(S).astype(np.float64)
    ce = gmax + np.log(gsum) - tlog
    w = (np.arange(S - 1) < valid_len - 1).astype(np.float64)
    loss = (ce[:S - 1] * w).sum() / w.sum()
    return np.float32(loss)



# revision 4
# speedup vs baseline: 1.3420x; 1.3420x over previous
"""Trainium2 Bass kernel for nn_Decoder_20486994002617.

8-core tensor-parallel 2-layer llama-style decoder with ragged token-merge
(handled on host), returning the masked-mean cross-entropy loss.

v2: big GEMMs (qkv / o / gate / up / down / lm_head) run in fp8e4 with
MatmulPerfMode.DoubleRow (2 contraction chunks per instruction).  Weights
are pre-scaled by 64 (16 for up-proj) on host so they sit in e4m3's normal
range; compensation is folded into the rope constants and into the
PSUM->SBUF output copies (scaled multiplies).  Weight matrices are
pre-chunked on host into the exact SBUF tile layout so every streaming DMA
is contiguous (2KB per partition).

Device layout choices (unchanged from v1):
  - h (residual) lives in SBUF as [128 part, 8 seq-tiles, 4096] bf16.
  - RMSNorm weights are folded into the consumer weight matrices on host.
  - Attention: 4 q-heads + 1 kv-head per core; scores/softmax in bf16.
  - MLP: intermediate dim sharded 1376/core, padded to 1408 (fp8 pairs pad
    the contraction to 1536 = 12*128 for the down-proj).
  - lm_head: vocab sharded 4000/core; softmax stats AllReduce'd; target
    logit via host-gathered column matrix (wsel, quantized with the same
    fp8 quantizer as lm_head for consistency).
Outputs per core: gmax [128,8] f32, gsum [128,8] f32, tlog [1,1024] f32.
Host finishes: ce = gmax + log(gsum) - tlog; loss = masked mean.
"""
import numpy as np
import ml_dtypes

from contextlib import ExitStack

import concourse.bass as bass
import concourse.bacc as bacc
import concourse.mybir as mybir
import concourse.tile as tile
from concourse.bass_utils import run_bass_kernel_spmd

F32 = mybir.dt.float32
BF16 = mybir.dt.bfloat16
FP8 = mybir.dt.float8e4
AF = mybir.ActivationFunctionType
ALU = mybir.AluOpType
AX = mybir.AxisListType
DR = mybir.MatmulPerfMode.DoubleRow

H, HD, NH, NKV = 4096, 128, 32, 8
L, V, S, I = 2, 32000, 1024, 11008
EPS, THETA = 1e-6, 10000.0
NC_ = 8          # cores
IPC = I // NC_   # 1376
IP = 1408        # padded intermediate per core = 11 * 128
IP2 = 1536       # fp8-pair-padded contraction for down proj = 12 * 128
VS = V // NC_    # 4000 vocab per core
NEG = -1e9
WS = 64.0        # fp8 weight scale (qkv, o, gate, down, lm_head)
US = 4.0         # fp8 weight scale for up-proj (y*US must stay under e4m3's 240)

bf16 = ml_dtypes.bfloat16
f8 = ml_dtypes.float8_e4m3

last_run_info = {}
_cache = {}


# ----------------------------------------------------------------- device --

def _norm_transpose(nc, pools, h_ap, dst, ident_sb, uid, nt_tag="nt_ps", nt_bufs=2):
    """dst[:, k, :] (32 chunks of [128,128]) = normalized transpose of
    h_ap ([128 seq rows, 4096]). dst free dims must be (32, 128)."""
    small, ntmp, psum = pools
    ssq = small.tile([128, 1], F32, tag="nt_ssq", bufs=2, name=f"ssq_{uid}")
    # Square scratch output goes into dst (overwritten by the transpose after)
    nc.scalar.activation(dst, h_ap.rearrange("p (k m) -> p k m", k=32),
                         AF.Square, accum_out=ssq[:])
    var = small.tile([128, 1], F32, tag="nt_var", bufs=2, name=f"var_{uid}")
    nc.vector.tensor_scalar(var[:], ssq[:], 1.0 / H, EPS, op0=ALU.mult, op1=ALU.add)
    std = small.tile([128, 1], F32, tag="nt_std", bufs=2, name=f"std_{uid}")
    nc.scalar.sqrt(std[:], var[:])
    fac = small.tile([128, 1], F32, tag="nt_fac", bufs=2, name=f"fac_{uid}")
    nc.vector.reciprocal(fac[:], std[:])
    diag = ntmp.tile([128, 128], BF16, tag="nt_diag", bufs=2, name=f"diag_{uid}")
    nc.vector.tensor_scalar_mul(diag[:], ident_sb[:], fac[:])
    for kk in range(8):
        pnt = psum.tile([128, 512], F32, tag=nt_tag, bufs=nt_bufs,
                        name=f"pnt_{uid}_{kk}")
        for j in range(4):
            k = kk * 4 + j
            nc.tensor.matmul(pnt[:, j * 128:(j + 1) * 128],
                             h_ap[:, k * 128:(k + 1) * 128], diag[:],
                             start=True, stop=True)
        nc.any.tensor_copy(dst[:, kk * 4:(kk + 1) * 4, :],
                           pnt[:].rearrange("p (j m) -> p j m", j=4))


def _rope(nc, pools, ps, cos_ap, sf_ap, out, nheads, i):
    """out (bf16 [128, nheads*128]) = rope(ps) with ps a psum slice."""
    small, ntmp, psum = pools
    n = nheads * 128
    t1 = ntmp.tile([128, 512], F32, tag="rope_t1", bufs=1, name=f"t1_{i}_{nheads}")
    t2 = ntmp.tile([128, 512], F32, tag="rope_t2", bufs=1, name=f"t2_{i}_{nheads}")
    nc.vector.tensor_mul(t1[:, :n], ps, cos_ap)
    for hh in range(nheads):
        b = hh * 128
        nc.vector.tensor_mul(t2[:, b:b + 64], ps[:, b + 64:b + 128],
                             sf_ap[:, b:b + 64])
        nc.vector.tensor_mul(t2[:, b + 64:b + 128], ps[:, b:b + 64],
                             sf_ap[:, b + 64:b + 128])
    nc.vector.tensor_add(out[:], t1[:, :n], t2[:, :n])


def build_nc():
    nc = bacc.Bacc("TRN2", target_bir_lowering=False, debug=False,
                   num_devices=NC_)

    din = {}
    def dram_in(name, shape, dtype=BF16):
        din[name] = nc.dram_tensor(name, shape, dtype, kind="ExternalInput")
        return din[name]

    h0_d = dram_in("h0", [S, H])
    cos4_d = dram_in("cos4", [S, 512])
    sf4_d = dram_in("sf4", [S, 512])
    ident_d = dram_in("ident", [128, 128])
    cmask_d = dram_in("cmask", [128, 128])
    ones_d = dram_in("ones", [128, 1])
    for l in range(L):
        # pre-chunked fp8 weights (SBUF tile layout, contiguous DMAs)
        dram_in(f"qkvw{l}", [128, 32, 768], FP8)
        dram_in(f"ow{l}", [128, 4, H], FP8)
        dram_in(f"gw{l}", [3, 8, 128, 4, 512], FP8)   # [nb, kp, p, j, n]
        dram_in(f"uw{l}", [3, 8, 128, 4, 512], FP8)
        dram_in(f"dw{l}", [8, 3, 128, 4, 512], FP8)   # [n, tp, p, j, n]
    lmw_d = dram_in("lmw", [8, 8, 128, 4, 500], FP8)  # [vb, kp, p, j, n]
    wsel_d = dram_in("wsel", [H, S])

    gmax_o = nc.dram_tensor("gmax_o", [128, 8], F32, kind="ExternalOutput")
    gsum_o = nc.dram_tensor("gsum_o", [128, 8], F32, kind="ExternalOutput")
    tlog_o = nc.dram_tensor("tlog_o", [1, S], F32, kind="ExternalOutput")

    rg = [list(range(NC_))]

    with tile.TileContext(nc) as tc:
        with (
            tc.tile_pool(name="pconst", bufs=1) as pconst,
            tc.tile_pool(name="psmall", bufs=1) as psmall,
            tc.tile_pool(name="pdram", bufs=1, space="DRAM") as pdram,
        ):
            ident_sb = pconst.tile([128, 128], BF16)
            cmask_sb = pconst.tile([128, 128], BF16)
            ones_sb = pconst.tile([128, 1], BF16)
            cos4_sb = pconst.tile([128, 8, 512], BF16)
            sf4_sb = pconst.tile([128, 8, 512], BF16)
            nc.sync.dma_start(ident_sb[:], ident_d.ap())
            nc.sync.dma_start(cmask_sb[:], cmask_d.ap())
            nc.sync.dma_start(ones_sb[:], ones_d.ap())
            for i in range(8):
                nc.sync.dma_start(cos4_sb[:, i, :], cos4_d.ap()[i * 128:(i + 1) * 128, :])
                nc.sync.dma_start(sf4_sb[:, i, :], sf4_d.ap()[i * 128:(i + 1) * 128, :])

            hstack = ExitStack()
            phh = hstack.enter_context(tc.tile_pool(name="phh", bufs=1))
            if True:
                h_sb = phh.tile([128, 8, H], BF16)
                for i in range(8):
                    nc.sync.dma_start(h_sb[:, i, :], h0_d.ap()[i * 128:(i + 1) * 128, :])

                ar_ins, ar_outss, ar2_ins, ar2_outss = [], [], [], []
                for l in range(L):
                    ar_ins.append(pdram.tile([S, H], BF16, tag=f"ar_in_{l}",
                                             name=f"ar_in_{l}"))
                    ar_outss.append([pdram.tile([512, H], BF16, addr_space="Shared",
                                                tag=f"ar_out_{l}_{c}",
                                                name=f"ar_out_{l}_{c}")
                                     for c in range(2)])
                    ar2_ins.append(pdram.tile([S, H], BF16, tag=f"ar2_in_{l}",
                                              name=f"ar2_in_{l}"))
                    ar2_outss.append([pdram.tile([512, H], BF16, addr_space="Shared",
                                                 tag=f"ar2_out_{l}_{c}",
                                                 name=f"ar2_out_{l}_{c}")
                                      for c in range(2)])

                for l in range(L):
                    # ======== attention: per-tile qkv -> heads -> o-proj ====
                    with (
                        tc.tile_pool(name="pal", bufs=1) as pal,
                        tc.tile_pool(name="paps", bufs=1, space="PSUM") as paps,
                    ):
                        kT_sb = pal.tile([128, S], BF16)
                        v_sb = pal.tile([128, 8, 128], BF16)
                        ar_in = ar_ins[l]
                        ar_outs = ar_outss[l]
                        pools = (psmall, pal, paps)
                        wqkv_sb = pal.tile([128, 32, 768], FP8)
                        ow_sb = pal.tile([128, 4, H], FP8)
                        nc.sync.dma_start(wqkv_sb[:], din[f"qkvw{l}"].ap())
                        nc.sync.dma_start(ow_sb[:], din[f"ow{l}"].ap())
                        for i in range(8):
                            if l > 0:
                                rt = pal.tile([128, H], BF16, tag="resprev",
                                              bufs=1, name=f"resprev_{l}_{i}")
                                nc.sync.dma_start(
                                    rt[:],
                                    ar2_outss[l - 1][i // 4][(i % 4) * 128:(i % 4 + 1) * 128, :])
                                nc.vector.tensor_add(h_sb[:, i, :], h_sb[:, i, :], rt[:])
                            qT_sb = pal.tile([128, 4, 128], BF16, tag="qT",
                                             bufs=2, name=f"qT_{l}_{i}")
                            oT_sb = pal.tile([128, 4, 128], FP8, tag="oT",
                                             bufs=2, name=f"oT_{l}_{i}")
                            xnt = pal.tile([128, 32, 128], FP8, tag="xnt",
                                           bufs=1, name=f"xnt_{l}_{i}")
                            _norm_transpose(nc, pools, h_sb[:, i, :], xnt, ident_sb,
                                            f"a{l}_{i}", nt_bufs=1)
                            psq = paps.tile([128, 512], F32, tag="psq", bufs=1,
                                            name=f"psq_{l}_{i}")
                            pskv = paps.tile([128, 256], F32, tag="pskv", bufs=1,
                                             name=f"pskv_{l}_{i}")
                            for k in range(16):
                                nc.tensor.matmul(psq[:], xnt[:, 2 * k:2 * k + 2, :],
                                                 wqkv_sb[:, 2 * k:2 * k + 2, 0:512],
                                                 start=(k == 0), stop=(k == 15),
                                                 perf_mode=DR)
                                nc.tensor.matmul(pskv[:], xnt[:, 2 * k:2 * k + 2, :],
                                                 wqkv_sb[:, 2 * k:2 * k + 2, 512:768],
                                                 start=(k == 0), stop=(k == 15),
                                                 perf_mode=DR)
                            q_rot = pal.tile([128, 512], BF16, tag="q_rot", bufs=2,
                                             name=f"qr_{l}_{i}")
                            k_rot = pal.tile([128, 128], BF16, tag="k_rot", bufs=2,
                                             name=f"kr_{l}_{i}")
                            _rope(nc, pools, psq[:], cos4_sb[:, i, :], sf4_sb[:, i, :],
                                  q_rot, 4, f"{l}_{i}")
                            _rope(nc, pools, pskv[:, 0:128], cos4_sb[:, i, 0:128],
                                  sf4_sb[:, i, 0:128], k_rot, 1, f"{l}_{i}")
                            nc.vector.tensor_scalar_mul(v_sb[:, i, :],
                                                        pskv[:, 128:256], 1.0 / WS)
                            for hh in range(4):
                                ptr = paps.tile([128, 512], F32, tag="mix", bufs=2,
                                                name=f"ptrq_{l}_{i}_{hh}")
                                nc.tensor.matmul(ptr[:, :128], q_rot[:, hh * 128:(hh + 1) * 128],
                                                 ident_sb[:], start=True, stop=True)
                                nc.any.tensor_copy(qT_sb[:, hh, :], ptr[:, :128])
                            ptrk = paps.tile([128, 512], F32, tag="mix", bufs=2,
                                             name=f"ptrk_{l}_{i}")
                            nc.tensor.matmul(ptrk[:, :128], k_rot[:], ident_sb[:],
                                             start=True, stop=True)
                            nc.any.tensor_copy(kT_sb[:, i * 128:(i + 1) * 128], ptrk[:, :128])
                            n2 = 128 * (i + 1)
                            for hh in range(4):
                                pss = paps.tile([128, 1024], F32, tag="pss", bufs=1,
                                                name=f"pss_{l}_{hh}_{i}")
                                lhs_q = qT_sb[:, hh, :]
                                c0 = 0
                                while c0 < n2 - 128:
                                    N = min(512, n2 - 128 - c0)
                                    nc.tensor.matmul(pss[:, c0:c0 + N], lhs_q,
                                                     kT_sb[:, c0:c0 + N],
                                                     start=True, stop=True)
                                    c0 += N
                                nc.tensor.matmul(pss[:, n2 - 128:n2], lhs_q,
                                                 kT_sb[:, n2 - 128:n2],
                                                 start=True, stop=False)
                                nc.tensor.matmul(pss[:, n2 - 128:n2], ident_sb[:],
                                                 cmask_sb[:], start=False, stop=True)
                                mx = psmall.tile([128, 1], F32, tag="mx", bufs=2,
                                                 name=f"mx_{l}_{hh}_{i}")
                                nc.vector.tensor_reduce(mx[:], pss[:, :n2], axis=AX.X,
                                                        op=ALU.max)
                                negm = psmall.tile([128, 1], F32, tag="negm", bufs=2,
                                                   name=f"negm_{l}_{hh}_{i}")
                                nc.vector.tensor_scalar_mul(negm[:], mx[:], -1.0)
                                sume = psmall.tile([128, 1], F32, tag="sume", bufs=2,
                                                   name=f"sume_{l}_{hh}_{i}")
                                exp_sb = pal.tile([128, 1024], BF16, tag="exp", bufs=1,
                                                  name=f"exp_{l}_{hh}_{i}")
                                nc.scalar.activation(exp_sb[:, :n2], pss[:, :n2], AF.Exp,
                                                     bias=negm[:], accum_out=sume[:])
                                rec = psmall.tile([128, 1], F32, tag="rec", bufs=2,
                                                  name=f"rec_{l}_{hh}_{i}")
                                nc.vector.reciprocal(rec[:], sume[:])
                                diag_r = pal.tile([128, 128], BF16, tag="diag_r", bufs=2,
                                                  name=f"diagr_{l}_{hh}_{i}")
                                nc.vector.tensor_scalar_mul(diag_r[:], ident_sb[:], rec[:])
                                atcol = pal.tile([128, 8, 128], BF16, tag="atcol", bufs=1,
                                                 name=f"atcol_{l}_{hh}_{i}")
                                for j in range(i + 1):
                                    pat = paps.tile([128, 512], F32, tag="mix", bufs=2,
                                                    name=f"pat_{l}_{hh}_{i}_{j}")
                                    nc.tensor.matmul(pat[:, :128], exp_sb[:, j * 128:(j + 1) * 128],
                                                     diag_r[:], start=True, stop=True)
                                    nc.any.tensor_copy(atcol[:, j, :], pat[:, :128])
                                pso = paps.tile([128, 128], F32, tag="pso", bufs=1,
                                                name=f"pso_{l}_{hh}_{i}")
                                for j in range(i + 1):
                                    nc.tensor.matmul(pso[:], v_sb[:, j, :], atcol[:, j, :],
                                                     start=(j == 0), stop=(j == i))
                                nc.any.tensor_copy(oT_sb[:, hh, :], pso[:])
                            ob = pal.tile([128, H], BF16, tag="ob", bufs=1,
                                          name=f"ob_{l}_{i}")
                            for n in range(8):
                                pps = paps.tile([128, 512], F32, tag="mix", bufs=2,
                                                name=f"pop_{l}_{i}_{n}")
                                for t in range(2):
                                    nc.tensor.matmul(pps[:], oT_sb[:, 2 * t:2 * t + 2, :],
                                                     ow_sb[:, 2 * t:2 * t + 2,
                                                           n * 512:(n + 1) * 512],
                                                     start=(t == 0), stop=(t == 1),
                                                     perf_mode=DR)
                                nc.vector.tensor_scalar_mul(ob[:, n * 512:(n + 1) * 512],
                                                            pps[:], 1.0 / WS)
                            nc.sync.dma_start(ar_in[i * 128:(i + 1) * 128, :], ob[:])
                            if i == 3:
                                nc.gpsimd.collective_compute(
                                    "AllReduce", ALU.add, replica_groups=rg,
                                    ins=[ar_in[0:512, :].opt()], outs=[ar_outs[0].opt()])
                        nc.gpsimd.collective_compute(
                            "AllReduce", ALU.add, replica_groups=rg,
                            ins=[ar_in[512:1024, :].opt()], outs=[ar_outs[1].opt()])

                    # ===== MLP: per-half gate/up -> down -> AR2 =============
                    with (
                        tc.tile_pool(name="pml", bufs=1) as pml,
                        tc.tile_pool(name="pmps", bufs=1, space="PSUM") as pmps,
                    ):
                        ar2_in = ar2_ins[l]
                        ar2_outs = ar2_outss[l]
                        for ig in range(2):
                            with tc.tile_pool(name="pgu", bufs=1) as pgu:
                                pools = (psmall, pgu, pmps)
                                yt_sb = pml.tile([128, 12, 512], FP8, tag="yt",
                                                 bufs=2, name=f"yt_{l}_{ig}")
                                nc.vector.memset(yt_sb[:, 11, :], 0.0)
                                xnts = []
                                for ii in range(4):
                                    i = ig * 4 + ii
                                    rt = pgu.tile([128, H], BF16, tag="resat",
                                                  bufs=1, name=f"resat_{l}_{i}")
                                    nc.sync.dma_start(
                                        rt[:], ar_outs[ig][ii * 128:(ii + 1) * 128, :])
                                    nc.vector.tensor_add(h_sb[:, i, :], h_sb[:, i, :], rt[:])
                                    xnt = pgu.tile([128, 32, 128], FP8, tag="xnt2",
                                                   bufs=4, name=f"xnt2_{l}_{i}")
                                    _norm_transpose(nc, pools, h_sb[:, i, :], xnt, ident_sb,
                                                    f"m{l}_{i}", nt_tag="mlpps", nt_bufs=4)
                                    xnts.append(xnt)
                                gu = {}
                                for wname, tag in ((f"gw{l}", "g"), (f"uw{l}", "u")):
                                    outs = [pgu.tile([128, IP], BF16, tag=tag, bufs=4,
                                                     name=f"{tag}_{l}_{ig}_{ii}")
                                            for ii in range(4)]
                                    gu[tag] = outs
                                    for nb in range(3):
                                        NB = 512 if nb < 2 else IP - 1024
                                        pg = [pmps.tile([128, 512], F32, tag="mlpps", bufs=4,
                                                        name=f"pg_{l}_{ig}_{tag}_{nb}_{ii}")
                                              for ii in range(4)]
                                        for kp in range(8):
                                            wt = pgu.tile([128, 4, 512], FP8, tag="wstream",
                                                          bufs=3,
                                                          name=f"wt_{l}_{ig}_{tag}_{nb}_{kp}")
                                            nc.sync.dma_start(
                                                wt[:], din[wname].ap()[nb, kp])
                                            for jp in range(2):
                                                k = kp * 4 + 2 * jp
                                                for ii in range(4):
                                                    nc.tensor.matmul(
                                                        pg[ii][:, :NB],
                                                        xnts[ii][:, k:k + 2, :],
                                                        wt[:, 2 * jp:2 * jp + 2, :NB],
                                                        start=(k == 0), stop=(k == 30),
                                                        perf_mode=DR)
                                        for ii in range(4):
                                            nc.any.tensor_copy(
                                                outs[ii][:, nb * 512:nb * 512 + NB],
                                                pg[ii][:, :NB])
                                for ii in range(4):
                                    i = ig * 4 + ii
                                    ysil = pgu.tile([128, IP], BF16, tag="ysil", bufs=2,
                                                    name=f"ysil_{l}_{i}")
                                    nc.scalar.activation(ysil[:], gu["g"][ii][:], AF.Silu,
                                                         scale=1.0 / WS)
                                    y = gu["u"][ii]
                                    nc.vector.tensor_mul(y[:], ysil[:], y[:])
                                    for tq in range(3):
                                        ts = [tq * 4 + j for j in range(4) if tq * 4 + j < 11]
                                        ptr = pmps.tile([128, 512], F32, tag="mlpps", bufs=4,
                                                        name=f"ytr_{l}_{i}_{tq}")
                                        for jj, t in enumerate(ts):
                                            nc.tensor.matmul(ptr[:, jj * 128:(jj + 1) * 128],
                                                             y[:, t * 128:(t + 1) * 128],
                                                             ident_sb[:], start=True, stop=True)
                                        nc.any.tensor_copy(
                                            yt_sb[:, ts[0]:ts[0] + len(ts),
                                                  ii * 128:(ii + 1) * 128],
                                            ptr[:, :len(ts) * 128].rearrange(
                                                "p (j m) -> p j m", j=len(ts)))
                                for n in range(8):
                                    pd = [pmps.tile([128, 512], F32, tag=f"pd{ii}", bufs=1,
                                                    name=f"pd_{l}_{ig}_{n}_{ii}")
                                          for ii in range(4)]
                                    for tp in range(3):
                                        dwt = pgu.tile([128, 4, 512], FP8, tag="dwstream",
                                                       bufs=3, name=f"dwt_{l}_{ig}_{n}_{tp}")
                                        nc.sync.dma_start(
                                            dwt[:], din[f"dw{l}"].ap()[n, tp])
                                        for jp in range(2):
                                            c = tp * 4 + 2 * jp
                                            for ii in range(4):
                                                nc.tensor.matmul(
                                                    pd[ii][:],
                                                    yt_sb[:, c:c + 2, ii * 128:(ii + 1) * 128],
                                                    dwt[:, 2 * jp:2 * jp + 2, :],
                                                    start=(c == 0), stop=(c == 10),
                                                    perf_mode=DR)
                                    for ii in range(4):
                                        i = ig * 4 + ii
                                        db = pgu.tile([128, 512], BF16, tag="db", bufs=4,
                                                      name=f"db_{l}_{ig}_{n}_{ii}")
                                        nc.vector.tensor_scalar_mul(db[:], pd[ii][:],
                                                                    1.0 / (WS * US))
                                        nc.sync.dma_start(
                                            ar2_in[i * 128:(i + 1) * 128,
                                                   n * 512:(n + 1) * 512], db[:])
                                nc.gpsimd.collective_compute(
                                    "AllReduce", ALU.add, replica_groups=rg,
                                    ins=[ar2_in[ig * 512:(ig + 1) * 512, :].opt()],
                                    outs=[ar2_outs[ig].opt()])

                # spill h to DRAM so the h pool can close before lm phase
                hdram = pdram.tile([S, H], BF16)
                for i in range(8):
                    nc.gpsimd.dma_start(hdram[i * 128:(i + 1) * 128, :], h_sb[:, i, :])
            hstack.close()  # release h pool

            # ======================= final norm -> xf ======================
            with tc.tile_pool(name="pxf", bufs=1) as pxf:
                xf_sb = pxf.tile([128, 32, S], FP8)
                with (
                    tc.tile_pool(name="pfn", bufs=1) as pfn,
                    tc.tile_pool(name="pfps", bufs=1, space="PSUM") as pfps,
                ):
                    pools = (psmall, pfn, pfps)
                    for i in range(8):
                        ht = pfn.tile([128, H], BF16, tag="hfin", bufs=2,
                                      name=f"hfin_{i}")
                        nc.gpsimd.dma_start(ht[:], hdram[i * 128:(i + 1) * 128, :])
                        rt = pfn.tile([128, H], BF16, tag="resfin", bufs=2,
                                      name=f"resfin_{i}")
                        nc.sync.dma_start(
                            rt[:], ar2_outss[L - 1][i // 4][(i % 4) * 128:(i % 4 + 1) * 128, :])
                        nc.vector.tensor_add(ht[:], ht[:], rt[:])
                        dst = xf_sb[:, :, i * 128:(i + 1) * 128]
                        _norm_transpose(nc, pools, ht[:], dst, ident_sb, f"f{i}")
                self_lm_phases(nc, tc, psmall, xf_sb, ident_sb, ones_sb,
                               wsel_d, lmw_d, tlog_o, gmax_o, gsum_o, rg, din)

    nc.compile()
    return nc


def self_lm_phases(nc, tc, psmall, xf_sb, ident_sb, ones_sb, wsel_d, lmw_d,
                   tlog_o, gmax_o, gsum_o, rg, din):
            if True:
                pass
            with (
                tc.tile_pool(name="ptl", bufs=1) as ptl,
                tc.tile_pool(name="ptps", bufs=1, space="PSUM") as ptps,
            ):
                pt0 = ptps.tile([1, 512], F32)
                pt1 = ptps.tile([1, 512], F32)
                for kp in range(8):
                    ws = ptl.tile([128, 4, S], BF16, tag="wsel", bufs=2, name=f"ws_{kp}")
                    nc.sync.dma_start(
                        ws[:], wsel_d.ap()[kp * 512:(kp + 1) * 512, :]
                        .rearrange("(j p) n -> p j n", p=128))
                    for jk in range(4):
                        k = kp * 4 + jk
                        tm = ptl.tile([128, S], BF16, tag="tm", bufs=2, name=f"tm_{k}")
                        nc.vector.tensor_mul(tm[:], xf_sb[:, k, :], ws[:, jk, :])
                        nc.tensor.matmul(pt0[:], ones_sb[:], tm[:, :512],
                                         start=(k == 0), stop=(k == 31))
                        nc.tensor.matmul(pt1[:], ones_sb[:], tm[:, 512:],
                                         start=(k == 0), stop=(k == 31))
                tl_sb = ptl.tile([1, S], F32)
                nc.any.tensor_copy(tl_sb[:, :512], pt0[:])
                nc.any.tensor_copy(tl_sb[:, 512:], pt1[:])
                nc.sync.dma_start(tlog_o.ap(), tl_sb[:])

            with (
                tc.tile_pool(name="plm", bufs=1) as plm,
                tc.tile_pool(name="plps", bufs=1, space="PSUM") as plps,
                tc.tile_pool(name="pld", bufs=1, space="DRAM") as pld,
            ):
                logits = [plm.tile([128, VS], BF16, tag=f"lg{i}", bufs=1,
                                   name=f"logits_{i}") for i in range(8)]
                for vb in range(8):
                    pl = [plps.tile([128, 500], F32, tag=f"pl{i}", bufs=1,
                                    name=f"pl_{vb}_{i}") for i in range(8)]
                    for kp in range(8):
                        lt = plm.tile([128, 4, 500], FP8, tag="lmw", bufs=4,
                                      name=f"lt_{vb}_{kp}")
                        nc.sync.dma_start(lt[:], lmw_d.ap()[vb, kp])
                        for jp in range(2):
                            k = kp * 4 + 2 * jp
                            for i in range(8):
                                nc.tensor.matmul(pl[i][:],
                                                 xf_sb[:, k:k + 2, i * 128:(i + 1) * 128],
                                                 lt[:, 2 * jp:2 * jp + 2, :],
                                                 start=(k == 0), stop=(k == 30),
                                                 perf_mode=DR)
                    for i in range(8):
                        nc.vector.tensor_scalar_mul(
                            logits[i][:, vb * 500:(vb + 1) * 500], pl[i][:], 1.0 / WS)

                gmax_sb = plm.tile([128, 8], F32)
                for i in range(8):
                    nc.vector.tensor_reduce(gmax_sb[:, i:i + 1], logits[i][:],
                                            axis=AX.X, op=ALU.max)
                gm_in = pld.tile([128, 8], F32)
                gm_out = pld.tile([128, 8], F32, addr_space="Shared")
                nc.sync.dma_start(gm_in[:], gmax_sb[:])
                nc.gpsimd.collective_compute("AllReduce", ALU.max, replica_groups=rg,
                                             ins=[gm_in.opt()], outs=[gm_out.opt()])
                gm_sb = plm.tile([128, 8], F32)
                nc.sync.dma_start(gm_sb[:], gm_out[:])
                nc.sync.dma_start(gmax_o.ap(), gm_sb[:])
                negg = plm.tile([128, 8], F32)
                nc.vector.tensor_scalar_mul(negg[:], gm_sb[:], -1.0)
                gs_sb = plm.tile([128, 8], F32)
                for i in range(8):
                    scr = plm.tile([128, VS], BF16, tag="scr", bufs=2, name=f"scr_{i}")
                    nc.scalar.activation(scr[:], logits[i][:], AF.Exp,
                                         bias=negg[:, i:i + 1],
                                         accum_out=gs_sb[:, i:i + 1])
                gs_in = pld.tile([128, 8], F32)
                gs_out = pld.tile([128, 8], F32, addr_space="Shared")
                nc.sync.dma_start(gs_in[:], gs_sb[:])
                nc.gpsimd.collective_compute("AllReduce", ALU.add, replica_groups=rg,
                                             ins=[gs_in.opt()], outs=[gs_out.opt()])
                gsf_sb = plm.tile([128, 8], F32)
                nc.sync.dma_start(gsf_sb[:], gs_out[:])
                nc.sync.dma_start(gsum_o.ap(), gsf_sb[:])


# ------------------------------------------------------------------- host --

def _to_f8(x):
    return np.clip(x, -240.0, 240.0).astype(f8)


def host_prep(inputs):
    inp = {k: np.asarray(v) for k, v in inputs.items()}
    embed = inp["embed"].astype(np.float32)
    ids = inp["input_ids"].reshape(-1).astype(np.int64)
    labels = inp["labels"].reshape(-1).astype(np.int64)

    h = embed[ids]
    cw = inp["conv_w"].astype(np.float32)
    logit = h[:-1] @ cw[0, :H] + h[1:] @ cw[0, H:] + np.float32(inp["conv_b"][0])
    mask = logit > 0
    m = np.concatenate([mask, [False]])
    hn = np.where(m[:, None], 0.5 * (h + np.roll(h, -1, axis=0)), h)
    keep = np.concatenate([[True], ~mask])
    order = np.argsort(~keep, kind="stable")
    h0 = hn[order]
    lab = labels[order]
    valid_len = int(keep.sum())

    inv = 1.0 / (THETA ** (np.arange(0, HD, 2, dtype=np.float32) / HD))
    t = np.arange(S, dtype=np.float32)
    freqs = np.outer(t, inv)
    emb = np.concatenate([freqs, freqs], -1)
    cos, sin = np.cos(emb), np.sin(emb)
    sinflip = np.concatenate([-sin[:, :HD // 2], sin[:, HD // 2:]], -1)
    # rope constants absorb the 1/WS compensation for the fp8 q/k weights
    cos4 = (np.tile(cos, (1, 4)) / WS).astype(bf16)
    sinflip4 = (np.tile(sinflip, (1, 4)) / WS).astype(bf16)

    ident = np.eye(128, dtype=bf16)
    cmask = np.where(np.arange(128)[None, :] > np.arange(128)[:, None],
                     np.float32(NEG), np.float32(0)).astype(bf16)
    ones = np.ones((128, 1), dtype=bf16)

    ln1 = inp["ln1_w"].astype(np.float32)
    ln2 = inp["ln2_w"].astype(np.float32)
    normw = inp["norm_w"].astype(np.float32)
    qsc = np.float32(1.0 / np.sqrt(HD))
    lm_folded = normw[:, None] * inp["lm_head_w"].astype(np.float32)
    lm_q = _to_f8(lm_folded * WS)          # quantized once, reused for wsel
    tgt = np.concatenate([lab[1:], [0]]).astype(np.int64)
    wsel = np.ascontiguousarray(lm_q.astype(np.float32)[:, tgt] / WS).astype(bf16)

    common = dict(h0=h0.astype(bf16), cos4=cos4, sf4=sinflip4, ident=ident,
                  cmask=cmask, ones=ones, wsel=wsel)
    in_maps = []
    for c in range(NC_):
        mcore = dict(common)
        for l in range(L):
            qw = ln1[l][:, None] * inp["q_w"][l].astype(np.float32) * qsc * WS
            kw = ln1[l][:, None] * inp["k_w"][l].astype(np.float32) * WS
            vw = ln1[l][:, None] * inp["v_w"][l].astype(np.float32) * WS
            gw = ln2[l][:, None] * inp["gate_w"][l].astype(np.float32) * WS
            uw = ln2[l][:, None] * inp["up_w"][l].astype(np.float32) * US
            dw = inp["down_w"][l].astype(np.float32) * WS
            qkv = np.concatenate(
                [qw[:, c * 512:(c + 1) * 512],
                 kw[:, c * 128:(c + 1) * 128],
                 vw[:, c * 128:(c + 1) * 128]], 1)          # [H, 768]
            # [H,768] -> [128, 32, 768] (partition p = row % 128, chunk k)
            mcore[f"qkvw{l}"] = np.ascontiguousarray(
                _to_f8(qkv).reshape(32, 128, 768).transpose(1, 0, 2))
            ow = inp["o_w"][l][c * 512:(c + 1) * 512].astype(np.float32) * WS
            mcore[f"ow{l}"] = np.ascontiguousarray(
                _to_f8(ow).reshape(4, 128, H).transpose(1, 0, 2))
            gws = np.zeros((H, IP), np.float32)
            uws = np.zeros((H, IP), np.float32)
            dws = np.zeros((IP2, H), np.float32)
            gws[:, :IPC] = gw[:, c * IPC:(c + 1) * IPC]
            uws[:, :IPC] = uw[:, c * IPC:(c + 1) * IPC]
            dws[:IPC] = dw[c * IPC:(c + 1) * IPC]
            # gate/up: [nb, kp, 128, 4, NB<=512] padded to 512 cols
            for wname, warr in ((f"gw{l}", gws), (f"uw{l}", uws)):
                out = np.zeros((3, 8, 128, 4, 512), np.float32)
                for nb in range(3):
                    NBc = 512 if nb < 2 else IP - 1024
                    blk = warr[:, nb * 512:nb * 512 + NBc]       # [H, NBc]
                    out[nb, :, :, :, :NBc] = blk.reshape(
                        8, 4, 128, NBc).transpose(0, 2, 1, 3)
                mcore[wname] = _to_f8(out)
            # down: [n, tp, 128, 4, 512]
            dout = np.zeros((8, 3, 128, 4, 512), np.float32)
            for n in range(8):
                blk = dws[:, n * 512:(n + 1) * 512]              # [IP2, 512]
                dout[n] = blk.reshape(3, 4, 128, 512).transpose(0, 2, 1, 3)
            mcore[f"dw{l}"] = _to_f8(dout)
        lmc = lm_q[:, c * VS:(c + 1) * VS].astype(np.float32)    # [H, 4000]
        lout = np.zeros((8, 8, 128, 4, 500), np.float32)
        for vb in range(8):
            blk = lmc[:, vb * 500:(vb + 1) * 500]                # [H, 500]
            lout[vb] = blk.reshape(8, 4, 128, 500).transpose(0, 2, 1, 3)
        mcore["lmw"] = _to_f8(lout)
        in_maps.append(mcore)

    return in_maps, valid_len


def kernel(**inputs) -> np.ndarray:
    in_maps, valid_len = host_prep(inputs)
    if "nc" not in _cache:
        _cache["nc"] = build_nc()
    nc = _cache["nc"]
    res = run_bass_kernel_spmd(nc, in_maps, list(range(NC_)),
                               **last_run_info.get("run_kwargs", {}))
    last_run_info["res"] = res
    out = res.results[0]
    gmax = out["gmax_o"].transpose(1, 0).reshape(S).astype(np.float64)
    gsum = out["gsum_o"].transpose(1, 0).reshape(S).astype(np.float64)
    tlog = out["tlog_o"].reshape(S).astype(np.float64)
    ce = gmax + np.log(gsum) - tlog
    w = (np.arange(S - 1) < valid_len - 1).astype(np.float64)
    loss = (ce[:S - 1] * w).sum() / w.sum()
    return np.float32(loss)


# revision 11
# speedup vs baseline: 1.4529x; 1.0827x over previous
"""Trainium2 Bass kernel for nn_Decoder_20486994002617.  v3.

8-core tensor-parallel 2-layer llama-style decoder with ragged token-merge
(handled on host), returning the masked-mean cross-entropy loss.

v2: fp8e4 DoubleRow for qkv / o / gate / up / down / lm_head, weights
pre-scaled (x64, up-proj x4) into e4m3 range, compensation folded into rope
constants and scaled PSUM->SBUF copies; host pre-chunks weights into SBUF
tile layout so streaming DMAs are contiguous.

v3 (latency restructure):
  - AllReduces quartered (256 rows each) and posted as soon as their rows
    are ready, so consumers never wait on a half-sequence collective.
  - A persistent "bridge" array xnt[0..7] holds the normalized transposed
    activations; the residual-add + rmsnorm + transpose for each phase is
    emitted inside the *previous* phase's instruction stream (prep), so
    GEMMs start immediately at phase entry.
  - lm head uses a fixed-max (M=16) online softmax: no logits storage, no
    max AllReduce, no DRAM spill of h.
  - tlog runs in sequence halves so it can start before the last xf tiles.
"""
import numpy as np
import ml_dtypes

from contextlib import ExitStack

import concourse.bass as bass
import concourse.bacc as bacc
import concourse.mybir as mybir
import concourse.tile as tile
from concourse.bass_utils import run_bass_kernel_spmd

F32 = mybir.dt.float32
BF16 = mybir.dt.bfloat16
FP8 = mybir.dt.float8e4
AF = mybir.ActivationFunctionType
ALU = mybir.AluOpType
AX = mybir.AxisListType
DR = mybir.MatmulPerfMode.DoubleRow

H, HD, NH, NKV = 4096, 128, 32, 8
L, V, S, I = 2, 32000, 1024, 11008
EPS, THETA = 1e-6, 10000.0
NC_ = 8          # cores
IPC = I // NC_   # 1376
IP = 1408        # padded intermediate per core = 11 * 128
IP2 = 1536       # fp8-pair-padded contraction for down proj = 12 * 128
VS = V // NC_    # 4000 vocab per core
NEG = -1e9
WS = 64.0        # fp8 weight scale (qkv, o, gate, down, lm_head)
US = 4.0         # fp8 weight scale for up-proj (y*US must stay under 240)
LM_MAX = 16.0    # fixed logsumexp shift (|logit| << 16)

bf16 = ml_dtypes.bfloat16
f8 = ml_dtypes.float8_e4m3

last_run_info = {}
_cache = {}


# ----------------------------------------------------------------- device --

def _norm_transpose(nc, small, ntmp, psum, h_ap, dst, ident_sb, uid,
                    nt_tag="mix", nt_bufs=2):
    """dst[:, k, :] (32 chunks of [128,128]) = normalized transpose of
    h_ap ([128 seq rows, 4096]). dst free dims must be (32, 128)."""
    ssq = small.tile([128, 1], F32, tag="nt_ssq", bufs=2, name=f"ssq_{uid}")
    # Square scratch output goes into dst (overwritten by the transpose after)
    nc.scalar.activation(dst, h_ap.rearrange("p (k m) -> p k m", k=32),
                         AF.Square, accum_out=ssq[:])
    var = small.tile([128, 1], F32, tag="nt_var", bufs=2, name=f"var_{uid}")
    nc.vector.tensor_scalar(var[:], ssq[:], 1.0 / H, EPS, op0=ALU.mult, op1=ALU.add)
    std = small.tile([128, 1], F32, tag="nt_std", bufs=2, name=f"std_{uid}")
    nc.scalar.sqrt(std[:], var[:])
    fac = small.tile([128, 1], F32, tag="nt_fac", bufs=2, name=f"fac_{uid}")
    nc.vector.reciprocal(fac[:], std[:])
    diag = ntmp.tile([128, 128], BF16, tag="nt_diag", bufs=2, name=f"diag_{uid}")
    nc.vector.tensor_scalar_mul(diag[:], ident_sb[:], fac[:])
    for kk in range(8):
        pnt = psum.tile([128, 512], F32, tag=nt_tag, bufs=nt_bufs,
                        name=f"pnt_{uid}_{kk}")
        for j in range(4):
            k = kk * 4 + j
            nc.tensor.matmul(pnt[:, j * 128:(j + 1) * 128],
                             h_ap[:, k * 128:(k + 1) * 128], diag[:],
                             start=True, stop=True)
        nc.any.tensor_copy(dst[:, kk * 4:(kk + 1) * 4, :],
                           pnt[:].rearrange("p (j m) -> p j m", j=4))


def _rope(nc, ntmp, ps, cos_ap, sf_ap, out, nheads, i):
    """out (bf16 [128, nheads*128]) = rope(ps); cos_ap/sf_ap are [128,128]."""
    n = nheads * 128
    t1 = ntmp.tile([128, 512], F32, tag="rope_t1", bufs=1, name=f"t1_{i}_{nheads}")
    t2 = ntmp.tile([128, 512], F32, tag="rope_t2", bufs=1, name=f"t2_{i}_{nheads}")
    for hh in range(nheads):
        b = hh * 128
        nc.vector.tensor_mul(t1[:, b:b + 128], ps[:, b:b + 128], cos_ap)
        nc.vector.tensor_mul(t2[:, b:b + 64], ps[:, b + 64:b + 128],
                             sf_ap[:, 0:64])
        nc.vector.tensor_mul(t2[:, b + 64:b + 128], ps[:, b:b + 64],
                             sf_ap[:, 64:128])
    nc.vector.tensor_add(out[:], t1[:, :n], t2[:, :n])


def build_nc():
    nc = bacc.Bacc("TRN2", target_bir_lowering=False, debug=False,
                   num_devices=NC_)

    din = {}
    def dram_in(name, shape, dtype=BF16):
        din[name] = nc.dram_tensor(name, shape, dtype, kind="ExternalInput")
        return din[name]

    h0_d = dram_in("h0", [S, H])
    cos1_d = dram_in("cos1", [S, 128])
    sf1_d = dram_in("sf1", [S, 128])
    ident_d = dram_in("ident", [128, 128])
    cmask_d = dram_in("cmask", [128, 128])
    ones_d = dram_in("ones", [128, 1])
    for l in range(L):
        dram_in(f"qkvw{l}", [128, 32, 768], FP8)
        dram_in(f"ow{l}", [128, 4, H], FP8)
        dram_in(f"gw{l}", [3, 8, 128, 4, 512], FP8)   # [nb, kp, p, j, n]
        dram_in(f"uw{l}", [3, 8, 128, 4, 512], FP8)
        dram_in(f"dw{l}", [8, 3, 128, 4, 512], FP8)   # [n, tp, p, j, n]
    lmw_d = dram_in("lmw", [8, 8, 128, 4, 500], FP8)  # [vb, kp, p, j, n]
    wsel_d = dram_in("wsel", [H, S])

    gsum_o = nc.dram_tensor("gsum_o", [128, 8], F32, kind="ExternalOutput")
    tlog_o = nc.dram_tensor("tlog_o", [1, S], F32, kind="ExternalOutput")

    rg = [list(range(NC_))]

    with tile.TileContext(nc) as tc:
        with (
            tc.tile_pool(name="pconst", bufs=1) as pconst,
            tc.tile_pool(name="psmall", bufs=1) as psmall,
            tc.tile_pool(name="pbridge", bufs=1) as pbridge,
            tc.tile_pool(name="pdram", bufs=1, space="DRAM") as pdram,
        ):
            ident_sb = pconst.tile([128, 128], BF16)
            cmask_sb = pconst.tile([128, 128], BF16)
            ones_sb = pconst.tile([128, 1], BF16)
            cos_sb = pconst.tile([128, 8, 128], BF16)
            sf_sb = pconst.tile([128, 8, 128], BF16)
            nc.sync.dma_start(ident_sb[:], ident_d.ap())
            nc.sync.dma_start(cmask_sb[:], cmask_d.ap())
            nc.sync.dma_start(ones_sb[:], ones_d.ap())
            for i in range(8):
                nc.sync.dma_start(cos_sb[:, i, :], cos1_d.ap()[i * 128:(i + 1) * 128, :])
                nc.sync.dma_start(sf_sb[:, i, :], sf1_d.ap()[i * 128:(i + 1) * 128, :])

            xnt = [pbridge.tile([128, 32, 128], FP8, name=f"xnt_{j}")
                   for j in range(8)]

            hstack = ExitStack()
            phh = hstack.enter_context(tc.tile_pool(name="phh", bufs=1))
            h_sb = phh.tile([128, 8, H], BF16)
            for i in range(8):
                nc.sync.dma_start(h_sb[:, i, :], h0_d.ap()[i * 128:(i + 1) * 128, :])

            # quarter-grained AR buffers: [4 quarters][256, H]
            ar_ins, ar_outss, ar2_ins, ar2_outss = [], [], [], []
            for l in range(L):
                ar_ins.append(pdram.tile([S, H], BF16, name=f"ar_in_{l}"))
                ar_outss.append([pdram.tile([256, H], BF16, addr_space="Shared",
                                            name=f"ar_out_{l}_{q}")
                                 for q in range(4)])
                ar2_ins.append(pdram.tile([S, H], BF16, name=f"ar2_in_{l}"))
                ar2_outss.append([pdram.tile([256, H], BF16, addr_space="Shared",
                                             name=f"ar2_out_{l}_{q}")
                                  for q in range(4)])

            def prep(pool, psum, j, res_q, dst, uid, nt_tag="mix", nt_bufs=2):
                """h_sb[:,j] += AR-quarter residual; dst = norm-transpose."""
                if res_q is not None:
                    rt = pool.tile([128, H], BF16, tag="prep_rt", bufs=2,
                                   name=f"rt_{uid}")
                    nc.sync.dma_start(
                        rt[:], res_q[(j % 2) * 128:(j % 2 + 1) * 128, :])
                    nc.vector.tensor_add(h_sb[:, j, :], h_sb[:, j, :], rt[:])
                _norm_transpose(nc, psmall, pool, psum, h_sb[:, j, :], dst,
                                ident_sb, uid, nt_tag=nt_tag, nt_bufs=nt_bufs)

            xfstack = ExitStack()

            for l in range(L):
                # ======== attention: per-tile qkv -> heads -> o-proj ========
                with (
                    tc.tile_pool(name="pal", bufs=1) as pal,
                    tc.tile_pool(name="paps", bufs=1, space="PSUM") as paps,
                ):
                    kT_sb = pal.tile([128, S], BF16)
                    v_sb = pal.tile([128, 8, 128], BF16)
                    ar_in = ar_ins[l]
                    ar_outs = ar_outss[l]
                    wqkv_sb = pal.tile([128, 32, 768], FP8)
                    ow_sb = pal.tile([128, 4, H], FP8)
                    nc.sync.dma_start(wqkv_sb[:], din[f"qkvw{l}"].ap())
                    nc.sync.dma_start(ow_sb[:], din[f"ow{l}"].ap())
                    if l == 0:
                        for j in range(8):
                            prep(pal, paps, j, None, xnt[j], f"i{j}")
                    for i in range(8):
                        if l > 0 and i in (2, 3):
                            # last two prep slots for this layer's attention
                            # (ar2 q3 of the previous layer lands late)
                            j = i + 4
                            prep(pal, paps, j, ar2_outss[l - 1][j // 2],
                                 xnt[j], f"a{l}_{j}")
                        psq = paps.tile([128, 512], F32, tag="psq", bufs=1,
                                        name=f"psq_{l}_{i}")
                        pskv = paps.tile([128, 256], F32, tag="pskv", bufs=1,
                                         name=f"pskv_{l}_{i}")
                        for k in range(16):
                            nc.tensor.matmul(psq[:], xnt[i][:, 2 * k:2 * k + 2, :],
                                             wqkv_sb[:, 2 * k:2 * k + 2, 0:512],
                                             start=(k == 0), stop=(k == 15),
                                             perf_mode=DR)
                            nc.tensor.matmul(pskv[:], xnt[i][:, 2 * k:2 * k + 2, :],
                                             wqkv_sb[:, 2 * k:2 * k + 2, 512:768],
                                             start=(k == 0), stop=(k == 15),
                                             perf_mode=DR)
                        qT_sb = pal.tile([128, 4, 128], BF16, tag="qT",
                                         bufs=2, name=f"qT_{l}_{i}")
                        oT_sb = pal.tile([128, 4, 128], FP8, tag="oT",
                                         bufs=2, name=f"oT_{l}_{i}")
                        q_rot = pal.tile([128, 512], BF16, tag="q_rot", bufs=2,
                                         name=f"qr_{l}_{i}")
                        k_rot = pal.tile([128, 128], BF16, tag="k_rot", bufs=2,
                                         name=f"kr_{l}_{i}")
                        _rope(nc, pal, psq[:], cos_sb[:, i, :], sf_sb[:, i, :],
                              q_rot, 4, f"{l}_{i}")
                        _rope(nc, pal, pskv[:, 0:128], cos_sb[:, i, :],
                              sf_sb[:, i, :], k_rot, 1, f"{l}_{i}")
                        nc.vector.tensor_scalar_mul(v_sb[:, i, :],
                                                    pskv[:, 128:256], 1.0 / WS)
                        for hh in range(4):
                            ptr = paps.tile([128, 512], F32, tag="mix", bufs=2,
                                            name=f"ptrq_{l}_{i}_{hh}")
                            nc.tensor.matmul(ptr[:, :128], q_rot[:, hh * 128:(hh + 1) * 128],
                                             ident_sb[:], start=True, stop=True)
                            nc.any.tensor_copy(qT_sb[:, hh, :], ptr[:, :128])
                        ptrk = paps.tile([128, 512], F32, tag="mix", bufs=2,
                                         name=f"ptrk_{l}_{i}")
                        nc.tensor.matmul(ptrk[:, :128], k_rot[:], ident_sb[:],
                                         start=True, stop=True)
                        nc.any.tensor_copy(kT_sb[:, i * 128:(i + 1) * 128], ptrk[:, :128])
                        n2 = 128 * (i + 1)
                        for hh in range(4):
                            pss = paps.tile([128, 1024], F32, tag="pss", bufs=1,
                                            name=f"pss_{l}_{hh}_{i}")
                            lhs_q = qT_sb[:, hh, :]
                            c0 = 0
                            while c0 < n2 - 128:
                                N = min(512, n2 - 128 - c0)
                                nc.tensor.matmul(pss[:, c0:c0 + N], lhs_q,
                                                 kT_sb[:, c0:c0 + N],
                                                 start=True, stop=True)
                                c0 += N
                            nc.tensor.matmul(pss[:, n2 - 128:n2], lhs_q,
                                             kT_sb[:, n2 - 128:n2],
                                             start=True, stop=False)
                            nc.tensor.matmul(pss[:, n2 - 128:n2], ident_sb[:],
                                             cmask_sb[:], start=False, stop=True)
                            mx = psmall.tile([128, 1], F32, tag="mx", bufs=2,
                                             name=f"mx_{l}_{hh}_{i}")
                            nc.vector.tensor_reduce(mx[:], pss[:, :n2], axis=AX.X,
                                                    op=ALU.max)
                            negm = psmall.tile([128, 1], F32, tag="negm", bufs=2,
                                               name=f"negm_{l}_{hh}_{i}")
                            nc.vector.tensor_scalar_mul(negm[:], mx[:], -1.0)
                            sume = psmall.tile([128, 1], F32, tag="sume", bufs=2,
                                               name=f"sume_{l}_{hh}_{i}")
                            exp_sb = pal.tile([128, 1024], BF16, tag="exp", bufs=1,
                                              name=f"exp_{l}_{hh}_{i}")
                            nc.scalar.activation(exp_sb[:, :n2], pss[:, :n2], AF.Exp,
                                                 bias=negm[:], accum_out=sume[:])
                            rec = psmall.tile([128, 1], F32, tag="rec", bufs=2,
                                              name=f"rec_{l}_{hh}_{i}")
                            nc.vector.reciprocal(rec[:], sume[:])
                            diag_r = pal.tile([128, 128], BF16, tag="diag_r", bufs=2,
                                              name=f"diagr_{l}_{hh}_{i}")
                            nc.vector.tensor_scalar_mul(diag_r[:], ident_sb[:], rec[:])
                            atcol = pal.tile([128, 8, 128], BF16, tag="atcol", bufs=1,
                                             name=f"atcol_{l}_{hh}_{i}")
                            for j in range(i + 1):
                                pat = paps.tile([128, 512], F32, tag="mix", bufs=2,
                                                name=f"pat_{l}_{hh}_{i}_{j}")
                                nc.tensor.matmul(pat[:, :128], exp_sb[:, j * 128:(j + 1) * 128],
                                                 diag_r[:], start=True, stop=True)
                                nc.any.tensor_copy(atcol[:, j, :], pat[:, :128])
                            pso = paps.tile([128, 128], F32, tag="pso", bufs=1,
                                            name=f"pso_{l}_{hh}_{i}")
                            for j in range(i + 1):
                                nc.tensor.matmul(pso[:], v_sb[:, j, :], atcol[:, j, :],
                                                 start=(j == 0), stop=(j == i))
                            nc.any.tensor_copy(oT_sb[:, hh, :], pso[:])
                        ob = pal.tile([128, H], BF16, tag="ob", bufs=1,
                                      name=f"ob_{l}_{i}")
                        for n in range(8):
                            pps = paps.tile([128, 512], F32, tag="mix", bufs=2,
                                            name=f"pop_{l}_{i}_{n}")
                            for t in range(2):
                                nc.tensor.matmul(pps[:], oT_sb[:, 2 * t:2 * t + 2, :],
                                                 ow_sb[:, 2 * t:2 * t + 2,
                                                       n * 512:(n + 1) * 512],
                                                 start=(t == 0), stop=(t == 1),
                                                 perf_mode=DR)
                            nc.vector.tensor_scalar_mul(ob[:, n * 512:(n + 1) * 512],
                                                        pps[:], 1.0 / WS)
                        nc.sync.dma_start(ar_in[i * 128:(i + 1) * 128, :], ob[:])
                        if i % 2 == 1:
                            q = i // 2
                            nc.gpsimd.collective_compute(
                                "AllReduce", ALU.add, replica_groups=rg,
                                ins=[ar_in[q * 256:(q + 1) * 256, :].opt()],
                                outs=[ar_outs[q].opt()])
                        if i >= 4:
                            # prep MLP tiles 0..3 (attn AR quarters 0,1 ready)
                            j = i - 4
                            prep(pal, paps, j, ar_outs[j // 2], xnt[j],
                                 f"m{l}_{j}")
                    # prep MLP tiles 4,5 (quarter 2 posted after i==5)
                    for j in (4, 5):
                        prep(pal, paps, j, ar_outs[j // 2], xnt[j], f"m{l}_{j}")

                # ===== MLP: gate/up -> down in row-quarters -> AR2 ==========
                if l == L - 1:
                    pxf = xfstack.enter_context(tc.tile_pool(name="pxf", bufs=1))
                    xf_sb = pxf.tile([128, 32, S], FP8)
                with (
                    tc.tile_pool(name="pml", bufs=1) as pml,
                    tc.tile_pool(name="pmps", bufs=1, space="PSUM") as pmps,
                ):
                    ar2_in = ar2_ins[l]
                    ar2_outs = ar2_outss[l]

                    def prep_next(j):
                        """prep for the next phase: attention l+1 (into xnt)
                        or the final norm (into xf)."""
                        if l < L - 1:
                            prep(pml, pmps, j, ar2_outs[j // 2], xnt[j],
                                 f"a{l + 1}_{j}", nt_tag="mlpps", nt_bufs=4)
                        else:
                            prep(pml, pmps, j, ar2_outs[j // 2],
                                 xf_sb[:, :, j * 128:(j + 1) * 128],
                                 f"f{j}", nt_tag="mlpps", nt_bufs=4)

                    for ig in range(2):
                        with tc.tile_pool(name="pgu", bufs=1) as pgu:
                            if ig == 1:
                                # prep MLP tiles 6,7 (attn AR q3 long done)
                                for j in (6, 7):
                                    prep(pml, pmps, j, ar_outs[j // 2], xnt[j],
                                         f"m{l}_{j}", nt_tag="mlpps", nt_bufs=4)
                            yt_sb = pml.tile([128, 12, 512], FP8, tag="yt",
                                             bufs=2, name=f"yt_{l}_{ig}")
                            nc.vector.memset(yt_sb[:, 11, :], 0.0)
                            gu = {}
                            for wi, (wname, tag) in enumerate(
                                    ((f"gw{l}", "g"), (f"uw{l}", "u"))):
                                outs = [pgu.tile([128, IP], BF16, tag=tag, bufs=4,
                                                 name=f"{tag}_{l}_{ig}_{ii}")
                                        for ii in range(4)]
                                gu[tag] = outs
                                for nb in range(3):
                                    NB = 512 if nb < 2 else IP - 1024
                                    pg = [pmps.tile([128, 512], F32, tag="mlpps", bufs=4,
                                                    name=f"pg_{l}_{ig}_{tag}_{nb}_{ii}")
                                          for ii in range(4)]
                                    for kp in range(8):
                                        wt = pgu.tile([128, 4, 512], FP8, tag="wstream",
                                                      bufs=3,
                                                      name=f"wt_{l}_{ig}_{tag}_{nb}_{kp}")
                                        nc.sync.dma_start(wt[:], din[wname].ap()[nb, kp])
                                        for jp in range(2):
                                            k = kp * 4 + 2 * jp
                                            for ii in range(4):
                                                nc.tensor.matmul(
                                                    pg[ii][:, :NB],
                                                    xnt[ig * 4 + ii][:, k:k + 2, :],
                                                    wt[:, 2 * jp:2 * jp + 2, :NB],
                                                    start=(k == 0), stop=(k == 30),
                                                    perf_mode=DR)
                                    for ii in range(4):
                                        nc.any.tensor_copy(
                                            outs[ii][:, nb * 512:nb * 512 + NB],
                                            pg[ii][:, :NB])
                                    # interleave next-phase preps into ig=1
                                    if ig == 1 and wi == 0:
                                        prep_next(nb)          # tiles 0,1,2
                                    if ig == 1 and wi == 1 and nb == 0:
                                        prep_next(3)
                            for ii in range(4):
                                i = ig * 4 + ii
                                ysil = pgu.tile([128, IP], BF16, tag="ysil", bufs=2,
                                                name=f"ysil_{l}_{i}")
                                nc.scalar.activation(ysil[:], gu["g"][ii][:], AF.Silu,
                                                     scale=1.0 / WS)
                                y = gu["u"][ii]
                                nc.vector.tensor_mul(y[:], ysil[:], y[:])
                                for tq in range(3):
                                    ts = [tq * 4 + j for j in range(4) if tq * 4 + j < 11]
                                    ptr = pmps.tile([128, 512], F32, tag="mlpps", bufs=4,
                                                    name=f"ytr_{l}_{i}_{tq}")
                                    for jj, t in enumerate(ts):
                                        nc.tensor.matmul(ptr[:, jj * 128:(jj + 1) * 128],
                                                         y[:, t * 128:(t + 1) * 128],
                                                         ident_sb[:], start=True, stop=True)
                                    nc.any.tensor_copy(
                                        yt_sb[:, ts[0]:ts[0] + len(ts),
                                              ii * 128:(ii + 1) * 128],
                                        ptr[:, :len(ts) * 128].rearrange(
                                            "p (j m) -> p j m", j=len(ts)))
                            for iq in range(2):
                                for n in range(8):
                                    pd = [pmps.tile([128, 512], F32, tag=f"pd{i2}", bufs=1,
                                                    name=f"pd_{l}_{ig}_{iq}_{n}_{i2}")
                                          for i2 in range(2)]
                                    for tp in range(3):
                                        dwt = pgu.tile([128, 4, 512], FP8, tag="dwstream",
                                                       bufs=3,
                                                       name=f"dwt_{l}_{ig}_{iq}_{n}_{tp}")
                                        nc.sync.dma_start(dwt[:], din[f"dw{l}"].ap()[n, tp])
                                        for jp in range(2):
                                            c = tp * 4 + 2 * jp
                                            for i2 in range(2):
                                                ii = iq * 2 + i2
                                                nc.tensor.matmul(
                                                    pd[i2][:],
                                                    yt_sb[:, c:c + 2,
                                                          ii * 128:(ii + 1) * 128],
                                                    dwt[:, 2 * jp:2 * jp + 2, :],
                                                    start=(c == 0), stop=(c == 10),
                                                    perf_mode=DR)
                                    for i2 in range(2):
                                        i = ig * 4 + iq * 2 + i2
                                        db = pgu.tile([128, 512], BF16, tag="db", bufs=2,
                                                      name=f"db_{l}_{ig}_{iq}_{n}_{i2}")
                                        nc.vector.tensor_scalar_mul(db[:], pd[i2][:],
                                                                    1.0 / (WS * US))
                                        nc.sync.dma_start(
                                            ar2_in[i * 128:(i + 1) * 128,
                                                   n * 512:(n + 1) * 512], db[:])
                                q = ig * 2 + iq
                                nc.gpsimd.collective_compute(
                                    "AllReduce", ALU.add, replica_groups=rg,
                                    ins=[ar2_in[q * 256:(q + 1) * 256, :].opt()],
                                    outs=[ar2_outs[q].opt()])
                            if ig == 1:
                                # tiles 4,5 of the next phase (q2 posted at iq0)
                                prep_next(4)
                                prep_next(5)
                    if l == L - 1:
                        # final xf tiles 6,7 (ar2 q3 just posted; short tail wait)
                        prep_next(6)
                        prep_next(7)

            # ==================== tlog + lm (online softmax) ================
            with (
                tc.tile_pool(name="ptl", bufs=1) as ptl,
                tc.tile_pool(name="ptps", bufs=1, space="PSUM") as ptps,
            ):
                pts = [ptps.tile([1, 512], F32, name=f"pt{h_}") for h_ in range(2)]
                for half in range(2):
                    for kp in range(8):
                        ws = ptl.tile([128, 4, 512], BF16, tag="wsel", bufs=2,
                                      name=f"ws_{half}_{kp}")
                        nc.sync.dma_start(
                            ws[:], wsel_d.ap()[kp * 512:(kp + 1) * 512,
                                               half * 512:(half + 1) * 512]
                            .rearrange("(j p) n -> p j n", p=128))
                        for jk in range(4):
                            k = kp * 4 + jk
                            tm = ptl.tile([128, 512], BF16, tag="tm", bufs=2,
                                          name=f"tm_{half}_{k}")
                            nc.vector.tensor_mul(
                                tm[:], xf_sb[:, k, half * 512:(half + 1) * 512],
                                ws[:, jk, :])
                            nc.tensor.matmul(pts[half][:], ones_sb[:], tm[:],
                                             start=(k == 0), stop=(k == 31))
                tl_sb = ptl.tile([1, S], F32)
                nc.any.tensor_copy(tl_sb[:, :512], pts[0][:])
                nc.any.tensor_copy(tl_sb[:, 512:], pts[1][:])
                nc.sync.dma_start(tlog_o.ap(), tl_sb[:])

            with (
                tc.tile_pool(name="plm", bufs=1) as plm,
                tc.tile_pool(name="plps", bufs=1, space="PSUM") as plps,
                tc.tile_pool(name="pld", bufs=1, space="DRAM") as pld,
            ):
                s_sb = plm.tile([128, 8], F32)
                nc.any.memset(s_sb[:], 0.0)
                negM = plm.tile([128, 1], F32)
                nc.any.memset(negM[:], -LM_MAX)
                for vb in range(8):
                    pl = [plps.tile([128, 500], F32, tag=f"pl{i}", bufs=1,
                                    name=f"pl_{vb}_{i}") for i in range(8)]
                    for kp in range(8):
                        lt = plm.tile([128, 4, 500], FP8, tag="lmw", bufs=4,
                                      name=f"lt_{vb}_{kp}")
                        nc.sync.dma_start(lt[:], lmw_d.ap()[vb, kp])
                        for jp in range(2):
                            k = kp * 4 + 2 * jp
                            for i in range(8):
                                nc.tensor.matmul(pl[i][:],
                                                 xf_sb[:, k:k + 2, i * 128:(i + 1) * 128],
                                                 lt[:, 2 * jp:2 * jp + 2, :],
                                                 start=(k == 0), stop=(k == 30),
                                                 perf_mode=DR)
                    for i in range(8):
                        se = psmall.tile([128, 1], F32, tag="se", bufs=2,
                                         name=f"se_{vb}_{i}")
                        scr = plm.tile([128, 500], BF16, tag="scr", bufs=2,
                                       name=f"scr_{vb}_{i}")
                        nc.scalar.activation(scr[:], pl[i][:], AF.Exp,
                                             bias=negM[:], scale=1.0 / WS,
                                             accum_out=se[:])
                        nc.vector.tensor_add(s_sb[:, i:i + 1], s_sb[:, i:i + 1],
                                             se[:])
                gs_in = pld.tile([128, 8], F32)
                gs_out = pld.tile([128, 8], F32, addr_space="Shared")
                nc.sync.dma_start(gs_in[:], s_sb[:])
                nc.gpsimd.collective_compute("AllReduce", ALU.add, replica_groups=rg,
                                             ins=[gs_in.opt()], outs=[gs_out.opt()])
                gsf_sb = plm.tile([128, 8], F32)
                nc.sync.dma_start(gsf_sb[:], gs_out[:])
                nc.sync.dma_start(gsum_o.ap(), gsf_sb[:])
            xfstack.close()
            hstack.close()

    nc.compile()
    return nc


# ------------------------------------------------------------------- host --

def _to_f8(x):
    return np.clip(x, -240.0, 240.0).astype(f8)


def host_prep(inputs):
    inp = {k: np.asarray(v) for k, v in inputs.items()}
    embed = inp["embed"].astype(np.float32)
    ids = inp["input_ids"].reshape(-1).astype(np.int64)
    labels = inp["labels"].reshape(-1).astype(np.int64)

    h = embed[ids]
    cw = inp["conv_w"].astype(np.float32)
    logit = h[:-1] @ cw[0, :H] + h[1:] @ cw[0, H:] + np.float32(inp["conv_b"][0])
    mask = logit > 0
    m = np.concatenate([mask, [False]])
    hn = np.where(m[:, None], 0.5 * (h + np.roll(h, -1, axis=0)), h)
    keep = np.concatenate([[True], ~mask])
    order = np.argsort(~keep, kind="stable")
    h0 = hn[order]
    lab = labels[order]
    valid_len = int(keep.sum())

    inv = 1.0 / (THETA ** (np.arange(0, HD, 2, dtype=np.float32) / HD))
    t = np.arange(S, dtype=np.float32)
    freqs = np.outer(t, inv)
    emb = np.concatenate([freqs, freqs], -1)
    cos, sin = np.cos(emb), np.sin(emb)
    sinflip = np.concatenate([-sin[:, :HD // 2], sin[:, HD // 2:]], -1)
    # rope constants absorb the 1/WS compensation for the fp8 q/k weights
    cos1 = (cos / WS).astype(bf16)
    sf1 = (sinflip / WS).astype(bf16)

    ident = np.eye(128, dtype=bf16)
    cmask = np.where(np.arange(128)[None, :] > np.arange(128)[:, None],
                     np.float32(NEG), np.float32(0)).astype(bf16)
    ones = np.ones((128, 1), dtype=bf16)

    ln1 = inp["ln1_w"].astype(np.float32)
    ln2 = inp["ln2_w"].astype(np.float32)
    normw = inp["norm_w"].astype(np.float32)
    qsc = np.float32(1.0 / np.sqrt(HD))
    lm_folded = normw[:, None] * inp["lm_head_w"].astype(np.float32)
    lm_q = _to_f8(lm_folded * WS)          # quantized once, reused for wsel
    tgt = np.concatenate([lab[1:], [0]]).astype(np.int64)
    wsel = np.ascontiguousarray(lm_q.astype(np.float32)[:, tgt] / WS).astype(bf16)

    common = dict(h0=h0.astype(bf16), cos1=cos1, sf1=sf1, ident=ident,
                  cmask=cmask, ones=ones, wsel=wsel)
    in_maps = []
    for c in range(NC_):
        mcore = dict(common)
        for l in range(L):
            qw = ln1[l][:, None] * inp["q_w"][l].astype(np.float32) * qsc * WS
            kw = ln1[l][:, None] * inp["k_w"][l].astype(np.float32) * WS
            vw = ln1[l][:, None] * inp["v_w"][l].astype(np.float32) * WS
            gw = ln2[l][:, None] * inp["gate_w"][l].astype(np.float32) * WS
            uw = ln2[l][:, None] * inp["up_w"][l].astype(np.float32) * US
            dw = inp["down_w"][l].astype(np.float32) * WS
            qkv = np.concatenate(
                [qw[:, c * 512:(c + 1) * 512],
                 kw[:, c * 128:(c + 1) * 128],
                 vw[:, c * 128:(c + 1) * 128]], 1)          # [H, 768]
            mcore[f"qkvw{l}"] = np.ascontiguousarray(
                _to_f8(qkv).reshape(32, 128, 768).transpose(1, 0, 2))
            ow = inp["o_w"][l][c * 512:(c + 1) * 512].astype(np.float32) * WS
            mcore[f"ow{l}"] = np.ascontiguousarray(
                _to_f8(ow).reshape(4, 128, H).transpose(1, 0, 2))
            gws = np.zeros((H, IP), np.float32)
            uws = np.zeros((H, IP), np.float32)
            dws = np.zeros((IP2, H), np.float32)
            gws[:, :IPC] = gw[:, c * IPC:(c + 1) * IPC]
            uws[:, :IPC] = uw[:, c * IPC:(c + 1) * IPC]
            dws[:IPC] = dw[c * IPC:(c + 1) * IPC]
            for wname, warr in ((f"gw{l}", gws), (f"uw{l}", uws)):
                out = np.zeros((3, 8, 128, 4, 512), np.float32)
                for nb in range(3):
                    NBc = 512 if nb < 2 else IP - 1024
                    blk = warr[:, nb * 512:nb * 512 + NBc]       # [H, NBc]
                    out[nb, :, :, :, :NBc] = blk.reshape(
                        8, 4, 128, NBc).transpose(0, 2, 1, 3)
                mcore[wname] = _to_f8(out)
            dout = np.zeros((8, 3, 128, 4, 512), np.float32)
            for n in range(8):
                blk = dws[:, n * 512:(n + 1) * 512]              # [IP2, 512]
                dout[n] = blk.reshape(3, 4, 128, 512).transpose(0, 2, 1, 3)
            mcore[f"dw{l}"] = _to_f8(dout)
        lmc = lm_q[:, c * VS:(c + 1) * VS].astype(np.float32)    # [H, 4000]
        lout = np.zeros((8, 8, 128, 4, 500), np.float32)
        for vb in range(8):
            blk = lmc[:, vb * 500:(vb + 1) * 500]                # [H, 500]
            lout[vb] = blk.reshape(8, 4, 128, 500).transpose(0, 2, 1, 3)
        mcore["lmw"] = _to_f8(lout)
        in_maps.append(mcore)

    return in_maps, valid_len


def kernel(**inputs) -> np.ndarray:
    in_maps, valid_len = host_prep(inputs)
    if "nc" not in _cache:
        _cache["nc"] = build_nc()
    nc = _cache["nc"]
    res = run_bass_kernel_spmd(nc, in_maps, list(range(NC_)),
                               **last_run_info.get("run_kwargs", {}))
    last_run_info["res"] = res
    out = res.results[0]
    gsum = out["gsum_o"].transpose(1, 0).reshape(S).astype(np.float64)
    tlog = out["tlog_o"].reshape(S).astype(np.float64)
    ce = LM_MAX + np.log(gsum) - tlog
    w = (np.arange(S - 1) < valid_len - 1).astype(np.float64)
    loss = (ce[:S - 1] * w).sum() / w.sum()
    return np.float32(loss)
